# revision 4
# baseline (speedup 1.0000x reference)
"""Trainium2 Bass kernel for Llama GQA attention (B=2, S=2048, H=4096,
32 Q heads / 8 KV heads, head_dim 128, RoPE, causal).

Sharding: tensor-parallel by head across 8 cores. Core c owns Q heads
[4c..4c+3] and KV head c. Each core computes its Q/K/V projections,
RoPE, causal attention, and a partial output projection over its 512
attention features; the host sums the 8 partial outputs.

Device layout is feature-major ([feature, token]) throughout:
  - QKV proj:  Q'[f,t] (psum) = sum_h WqT[h,f].T @ xT[h,t]   (f32r)
  - RoPE:      q*cos + swap_halves(q)*sign*sin               (DVE + DMA swap)
  - scores:    S.T[k,q] = K'[d,k].T @ Q'[d,q]                (f32r, N=512)
  - softmax:   exp on ACT (no max subtraction; scores are O(10)),
               denominator = ones[k,1].T @ E[k,q] matmul, reciprocal,
               broadcast via K=1 matmul, normalize fused into psum evict
  - AV:        U[d,q] = Vtok[k,d].T @ E[k,q]                 (bf16)
  - out:       out[t,o] = attn'[f,t].T @ WoT[f,o]            (f32r, partial)
"""
import math
import numpy as np
import ml_dtypes

import concourse.bacc as bacc
import concourse.tile as tile
from concourse import mybir
from concourse.bass_utils import run_bass_kernel_spmd

F32 = mybir.dt.float32
F32R = mybir.dt.float32r
BF16 = mybir.dt.bfloat16

P = 128
B, S, H = 2, 2048, 4096
T = B * S                    # 4096 tokens
DK = 128                     # head dim
NHL = 4                      # q heads per core
FL = NHL * DK                # 512 local q features
TB = 512                     # phase-1 token block
NTB = T // TB                # 8
NA = H // P                  # 32 contraction tiles
QBS = 512                    # attention q-block
NQB = S // QBS               # 4 q-blocks per (batch, head)
NKT = S // P                 # 16 k-tiles per batch
SCALE = 1.0 / math.sqrt(DK)
NOB = H // 512               # 8 output column blocks
NTT = T // P                 # 32 output row tiles

_NC_CACHE = {}


def build():
    nc = bacc.Bacc(None, target_bir_lowering=False)

    xt = nc.dram_tensor("xt", [H, T], F32R, kind="ExternalInput")
    wqt = nc.dram_tensor("wqt", [H, FL], F32R, kind="ExternalInput")
    wkt = nc.dram_tensor("wkt", [H, DK], F32R, kind="ExternalInput")
    wvt = nc.dram_tensor("wvt", [H, DK], F32R, kind="ExternalInput")
    wot = nc.dram_tensor("wot", [FL, H], F32R, kind="ExternalInput")
    cost = nc.dram_tensor("cost", [P, S], F32, kind="ExternalInput")
    sints = nc.dram_tensor("sints", [P, S], F32, kind="ExternalInput")
    trimask = nc.dram_tensor("trimask", [P, P], BF16, kind="ExternalInput")
    identb = nc.dram_tensor("identb", [P, P], BF16, kind="ExternalInput")
    onesc = nc.dram_tensor("onesc", [P, 1], BF16, kind="ExternalInput")
    onesr = nc.dram_tensor("onesr", [1, P], F32R, kind="ExternalInput")
    out = nc.dram_tensor("out", [T, H], F32, kind="ExternalOutput")

    EXP = mybir.ActivationFunctionType.Exp

    with nc.allow_low_precision(reason="f32r is 4-byte storage; adds round "
                                       "to f32r mantissa only"), \
         tile.TileContext(nc) as tc:
        with tc.tile_pool(name="const", bufs=1) as cp, \
             tc.tile_pool(name="dram", bufs=1, space="DRAM") as dp:
            cos_sb = cp.tile([P, S], F32)
            sin_sb = cp.tile([P, S], F32)
            tri_sb = cp.tile([P, P], BF16)
            id_sb = cp.tile([P, P], BF16)
            oc_sb = cp.tile([P, 1], BF16)
            or_sb = cp.tile([1, P], F32R)
            nc.sync.dma_start(out=cos_sb, in_=cost[:, :])
            nc.sync.dma_start(out=sin_sb, in_=sints[:, :])
            nc.sync.dma_start(out=tri_sb, in_=trimask[:, :])
            nc.sync.dma_start(out=id_sb, in_=identb[:, :])
            nc.sync.dma_start(out=oc_sb, in_=onesc[:, :])
            nc.sync.dma_start(out=or_sb, in_=onesr[:, :])

            q_scr = dp.tile([FL, T], F32R)       # Q' after rope, f-major
            k_scr = dp.tile([DK, T], F32R)       # K' after rope
            v_scr = dp.tile([T, DK], BF16)       # V token-major

            # ---------------- Phase 1: QKV projection + RoPE ----------------
            with tc.tile_pool(name="wq", bufs=1) as wqp, \
                 tc.tile_pool(name="xp", bufs=3) as xp, \
                 tc.tile_pool(name="rp", bufs=2) as rp, \
                 tc.tile_pool(name="ps1", bufs=1, space="PSUM") as ps1, \
                 tc.tile_pool(name="pst", bufs=2, space="PSUM") as pst:
                wq_sb = wqp.tile([P, NA * FL], F32R)      # 8 MB
                wk_sb = wqp.tile([P, NA * DK], F32R)      # 2 MB
                wv_sb = wqp.tile([P, NA * DK], F32R)      # 2 MB
                for a in range(NA):
                    nc.sync.dma_start(out=wq_sb[:, a * FL:(a + 1) * FL],
                                      in_=wqt[a * P:(a + 1) * P, :])
                    nc.sync.dma_start(out=wk_sb[:, a * DK:(a + 1) * DK],
                                      in_=wkt[a * P:(a + 1) * P, :])
                    nc.sync.dma_start(out=wv_sb[:, a * DK:(a + 1) * DK],
                                      in_=wvt[a * P:(a + 1) * P, :])

                for tb in range(NTB):
                    s0 = (tb * TB) % S
                    psq = [ps1.tile([P, TB], F32, name=f"psq{j}_{tb}",
                                    tag=f"psq{j}") for j in range(NHL)]
                    psk = ps1.tile([P, TB], F32, name=f"psk_{tb}", tag="psk")
                    psv = ps1.tile([P, TB], F32, name=f"psv_{tb}", tag="psv")
                    for a in range(NA):
                        xt_t = xp.tile([P, TB], F32R, name=f"x_{tb}_{a}",
                                       tag="xt")
                        nc.sync.dma_start(
                            out=xt_t,
                            in_=xt[a * P:(a + 1) * P, tb * TB:(tb + 1) * TB])
                        st, sp = (a == 0), (a == NA - 1)
                        nc.tensor.matmul(psk, wk_sb[:, a * DK:(a + 1) * DK],
                                         xt_t, start=st, stop=sp)
                        nc.tensor.matmul(psv, wv_sb[:, a * DK:(a + 1) * DK],
                                         xt_t, start=st, stop=sp)
                        for j in range(NHL):
                            nc.tensor.matmul(
                                psq[j],
                                wq_sb[:, a * FL + j * DK:a * FL + (j + 1) * DK],
                                xt_t, start=st, stop=sp)

                    # RoPE on the 4 q tiles + k tile, write to scratch
                    for src, scr, r0 in [(psq[0], q_scr, 0),
                                         (psq[1], q_scr, P),
                                         (psq[2], q_scr, 2 * P),
                                         (psq[3], q_scr, 3 * P),
                                         (psk, k_scr, 0)]:
                        qc = rp.tile([P, TB], F32, name=f"qc_{tb}_{r0}",
                                     tag="qc")
                        nc.scalar.copy(qc, src)
                        sw = rp.tile([P, TB], F32, name=f"sw_{tb}_{r0}",
                                     tag="sw")
                        nc.sync.dma_start(out=sw[0:64, :], in_=qc[64:128, :])
                        nc.sync.dma_start(out=sw[64:128, :], in_=qc[0:64, :])
                        t1 = rp.tile([P, TB], F32, name=f"t1_{tb}_{r0}",
                                     tag="t1")
                        nc.vector.tensor_mul(t1, src, cos_sb[:, s0:s0 + TB])
                        t2 = rp.tile([P, TB], F32, name=f"t2_{tb}_{r0}",
                                     tag="t2")
                        nc.vector.tensor_mul(t2, sw, sin_sb[:, s0:s0 + TB])
                        qf = rp.tile([P, TB], F32R, name=f"qf_{tb}_{r0}",
                                     tag="qf")
                        nc.vector.tensor_add(qf, t1, t2)
                        nc.sync.dma_start(
                            out=scr[r0:r0 + P, tb * TB:(tb + 1) * TB], in_=qf)

                    # V: cast to bf16, transpose to token-major, store
                    vb = rp.tile([P, TB], BF16, name=f"vb_{tb}", tag="vb")
                    nc.scalar.copy(vb, psv)
                    for u in range(TB // P):
                        vt_ps = pst.tile([P, P], BF16, name=f"vt_{tb}_{u}",
                                         tag="vtp")
                        nc.tensor.transpose(vt_ps, vb[:, u * P:(u + 1) * P],
                                            id_sb)
                        vt_sb = rp.tile([P, P], BF16, name=f"vs_{tb}_{u}",
                                        tag="vts")
                        nc.vector.tensor_copy(vt_sb, vt_ps)
                        nc.sync.dma_start(
                            out=v_scr[tb * TB + u * P:tb * TB + (u + 1) * P, :],
                            in_=vt_sb)

            # ---------------- Phase 2+3 pools ----------------
            with tc.tile_pool(name="attn", bufs=1) as ap:
                attn_sb = [ap.tile([P, T], F32R, name=f"attn{h}")
                           for h in range(NHL)]

                # ---------------- Phase 2: causal attention ----------------
                with tc.tile_pool(name="p2", bufs=1) as p2, \
                     tc.tile_pool(name="p2q", bufs=2) as p2q, \
                     tc.tile_pool(name="p2e", bufs=4) as p2e, \
                     tc.tile_pool(name="ps2s", bufs=2, space="PSUM") as ps2s, \
                     tc.tile_pool(name="ps2u", bufs=2, space="PSUM") as ps2u, \
                     tc.tile_pool(name="ps2r", bufs=2, space="PSUM") as ps2r:
                    for b in range(B):
                        kb_sb = p2q.tile([P, S], F32R, name=f"kb_{b}",
                                         tag="kb")
                        nc.sync.dma_start(out=kb_sb,
                                          in_=k_scr[:, b * S:(b + 1) * S])
                        vtk = p2q.tile([P, NKT, P], BF16, name=f"vt_{b}",
                                       tag="vtk")
                        nc.sync.dma_start(
                            out=vtk,
                            in_=v_scr[b * S:(b + 1) * S, :].rearrange(
                                "(n p) d -> p n d", p=P))
                        for h in range(NHL):
                            qh_sb = p2q.tile([P, S], F32R, name=f"q_{b}_{h}",
                                             tag="qh")
                            nc.sync.dma_start(
                                out=qh_sb,
                                in_=q_scr[h * P:(h + 1) * P, b * S:(b + 1) * S])
                            for qb in range(NQB):
                                nkt = 4 * qb + 4
                                u_ps = ps2u.tile([P, QBS], F32,
                                                 name=f"u_{b}_{h}_{qb}",
                                                 tag="u")
                                d_ps = ps2u.tile([1, QBS], F32,
                                                 name=f"d_{b}_{h}_{qb}",
                                                 tag="d")
                                for kt in range(nkt):
                                    s_ps = ps2s.tile([P, QBS], F32,
                                                     name=f"s_{b}_{h}_{qb}_{kt}",
                                                     tag="s")
                                    nc.tensor.matmul(
                                        s_ps,
                                        kb_sb[:, kt * P:(kt + 1) * P],
                                        qh_sb[:, qb * QBS:(qb + 1) * QBS],
                                        start=True, stop=True)
                                    e_sb = p2e.tile([P, QBS], BF16,
                                                    name=f"e_{b}_{h}_{qb}_{kt}",
                                                    tag="e")
                                    nc.scalar.activation(e_sb, s_ps, EXP,
                                                         scale=SCALE)
                                    m = kt - 4 * qb
                                    if m >= 0:
                                        if m > 0:
                                            nc.vector.memset(
                                                e_sb[:, 0:m * P], 0.0)
                                        nc.vector.tensor_mul(
                                            e_sb[:, m * P:(m + 1) * P],
                                            e_sb[:, m * P:(m + 1) * P],
                                            tri_sb)
                                    st, sp = (kt == 0), (kt == nkt - 1)
                                    nc.tensor.matmul(u_ps, vtk[:, kt, :],
                                                     e_sb, start=st, stop=sp)
                                    nc.tensor.matmul(d_ps, oc_sb, e_sb,
                                                     start=st, stop=sp)
                                r_sb = p2.tile([1, QBS], F32R,
                                               name=f"r_{b}_{h}_{qb}",
                                               tag="r", bufs=2)
                                nc.vector.reciprocal(r_sb, d_ps)
                                rb_ps = ps2r.tile([P, QBS], F32,
                                                  name=f"rb_{b}_{h}_{qb}",
                                                  tag="rb")
                                nc.tensor.matmul(rb_ps, or_sb, r_sb,
                                                 start=True, stop=True)
                                rb_sb = p2.tile([P, QBS], F32,
                                                name=f"rs_{b}_{h}_{qb}",
                                                tag="rs", bufs=2)
                                nc.scalar.copy(rb_sb, rb_ps)
                                nc.vector.tensor_mul(
                                    attn_sb[h][:, b * S + qb * QBS:
                                               b * S + (qb + 1) * QBS],
                                    u_ps, rb_sb)

                # ---------------- Phase 3: output projection ----------------
                with tc.tile_pool(name="p3w", bufs=2) as p3w, \
                     tc.tile_pool(name="p3o", bufs=4) as p3o, \
                     tc.tile_pool(name="ps3", bufs=4, space="PSUM") as ps3:
                    for ob in range(NOB):
                        wo_sb = p3w.tile([P, NHL, 512], F32R,
                                         name=f"wo_{ob}", tag="wo")
                        for j in range(NHL):
                            nc.sync.dma_start(
                                out=wo_sb[:, j, :],
                                in_=wot[j * P:(j + 1) * P,
                                        ob * 512:(ob + 1) * 512])
                        for tt in range(NTT):
                            o_ps = ps3.tile([P, 512], F32,
                                            name=f"o_{ob}_{tt}", tag="o")
                            for j in range(NHL):
                                nc.tensor.matmul(
                                    o_ps,
                                    attn_sb[j][:, tt * P:(tt + 1) * P],
                                    wo_sb[:, j, :],
                                    start=(j == 0), stop=(j == NHL - 1))
                            o_sb = p3o.tile([P, 512], F32,
                                            name=f"os_{ob}_{tt}", tag="os")
                            nc.scalar.copy(o_sb, o_ps)
                            nc.sync.dma_start(
                                out=out[tt * P:(tt + 1) * P,
                                        ob * 512:(ob + 1) * 512],
                                in_=o_sb)

    nc.compile()
    return nc


def _prep_inputs(hidden_states, Wq, Wk, Wv, Wo, cos, sin):
    hs = np.asarray(hidden_states, dtype=np.float32)
    Wq = np.asarray(Wq, dtype=np.float32)
    Wk = np.asarray(Wk, dtype=np.float32)
    Wv = np.asarray(Wv, dtype=np.float32)
    Wo = np.asarray(Wo, dtype=np.float32)
    cos = np.asarray(cos, dtype=np.float32)
    sin = np.asarray(sin, dtype=np.float32)

    xt = np.ascontiguousarray(hs.reshape(T, H).T)          # [H, T]
    cosT = np.ascontiguousarray(cos.T)                     # [128, S]
    sinT = np.ascontiguousarray(sin.T)
    sints = np.concatenate([-sinT[:64], sinT[64:]], axis=0)
    kq = np.arange(P)
    trim = (kq[None, :] >= kq[:, None]).astype(ml_dtypes.bfloat16)  # [k,q]
    ident = np.eye(P, dtype=ml_dtypes.bfloat16)
    onesc = np.ones((P, 1), dtype=ml_dtypes.bfloat16)
    onesr = np.ones((1, P), dtype=np.float32)

    in_maps = []
    for c in range(8):
        in_maps.append({
            "xt": xt,
            "wqt": np.ascontiguousarray(Wq[c * FL:(c + 1) * FL, :].T),
            "wkt": np.ascontiguousarray(Wk[c * DK:(c + 1) * DK, :].T),
            "wvt": np.ascontiguousarray(Wv[c * DK:(c + 1) * DK, :].T),
            "wot": np.ascontiguousarray(Wo[:, c * FL:(c + 1) * FL].T),
            "cost": cosT,
            "sints": np.ascontiguousarray(sints),
            "trimask": trim,
            "identb": ident,
            "onesc": onesc,
            "onesr": onesr,
        })
    return in_maps


def kernel(hidden_states, Wq, Wk, Wv, Wo, cos, sin, _run_kwargs=None):
    in_maps = _prep_inputs(hidden_states, Wq, Wk, Wv, Wo, cos, sin)
    if "nc" not in _NC_CACHE:
        _NC_CACHE["nc"] = build()
    nc = _NC_CACHE["nc"]
    kw = _run_kwargs or {}
    res = run_bass_kernel_spmd(nc, in_maps, core_ids=list(range(8)), **kw)
    acc = np.zeros((T, H), dtype=np.float64)
    for c in range(8):
        acc += np.asarray(res.results[c]["out"], dtype=np.float64)
    out = acc.astype(np.float32).reshape(B, S, H)
    if kw:
        _NC_CACHE["last_results"] = res
    return out


# revision 5
# speedup vs baseline: 1.0925x; 1.0925x over previous
"""Trainium2 Bass kernel for Llama GQA attention (B=2, S=2048, H=4096,
32 Q heads / 8 KV heads, head_dim 128, RoPE, causal).

Sharding: tensor-parallel by head across 8 cores. Core c owns Q heads
[4c..4c+3] and KV head c. Each core computes its Q/K/V projections,
RoPE, causal attention, and a partial output projection over its 512
attention features; the host sums the 8 partial outputs.

Device layout is feature-major ([feature, token]) throughout:
  - QKV proj:  Q'[f,t] (psum) = sum_h WqT[h,f].T @ xT[h,t]
  - RoPE:      q*cos + swap_halves(q)*sign*sin   (DVE + DMA swap)
  - scores:    S.T[k,q] = K'[d,k].T @ Q'[d,q]    (softmax over partition)
  - softmax:   exp on ACT (no max subtraction; scores are O(10)),
               denominator via ones-column matmul, reciprocal,
               broadcast via K=1 matmul, normalize on psum evict
  - AV:        U[d,q] = Vtok[k,d].T @ E[k,q]     (bf16)
  - out:       out[t,o] = attn'[f,t].T @ WoT[f,o]  (partial; host sums)
"""
import math
import numpy as np
import ml_dtypes

import concourse.bacc as bacc
import concourse.tile as tile
from concourse import mybir
from concourse.bass_utils import run_bass_kernel_spmd

F32 = mybir.dt.float32
F32R = mybir.dt.float32r
BF16 = mybir.dt.bfloat16

# Compute dtypes (bf16 matmuls stream at 1 cyc/row; f32r takes 2 passes)
DT_PROJ = BF16     # x / Wq / Wk / Wv and the QKV projection matmuls
DT_QK = BF16       # Q'/K' after rope -> scores matmul
DT_ATT = BF16      # attn' and WoT -> output projection matmul
NP_PROJ = ml_dtypes.bfloat16 if DT_PROJ == BF16 else np.float32
NP_ATT = ml_dtypes.bfloat16 if DT_ATT == BF16 else np.float32

P = 128
B, S, H = 2, 2048, 4096
T = B * S
DK = 128                     # head dim
NHL = 4                      # q heads per core
FL = NHL * DK                # 512 local q features
TB = 512                     # phase-1 token block
NTB = T // TB                # 8
NA = H // P                  # 32 contraction tiles
QBS = 512                    # attention q-block
NQB = S // QBS               # 4 q-blocks per (batch, head)
NKT = S // P                 # 16 k-tiles per batch
SCALE = 1.0 / math.sqrt(DK)
NOB = H // 512               # 8 output column blocks
NTT = T // P                 # 32 output row tiles

_NC_CACHE = {}


def build():
    nc = bacc.Bacc(None, target_bir_lowering=False)

    xt = nc.dram_tensor("xt", [H, T], DT_PROJ, kind="ExternalInput")
    wqt = nc.dram_tensor("wqt", [H, FL], DT_PROJ, kind="ExternalInput")
    wkt = nc.dram_tensor("wkt", [H, DK], DT_PROJ, kind="ExternalInput")
    wvt = nc.dram_tensor("wvt", [H, DK], DT_PROJ, kind="ExternalInput")
    wot = nc.dram_tensor("wot", [FL, H], DT_ATT, kind="ExternalInput")
    cost = nc.dram_tensor("cost", [P, S], F32, kind="ExternalInput")
    sints = nc.dram_tensor("sints", [P, S], F32, kind="ExternalInput")
    trimask = nc.dram_tensor("trimask", [P, P], BF16, kind="ExternalInput")
    identb = nc.dram_tensor("identb", [P, P], BF16, kind="ExternalInput")
    onesc = nc.dram_tensor("onesc", [P, 1], BF16, kind="ExternalInput")
    onesr = nc.dram_tensor("onesr", [1, P], F32R, kind="ExternalInput")
    out = nc.dram_tensor("out", [T, H], F32, kind="ExternalOutput")

    EXP = mybir.ActivationFunctionType.Exp

    with nc.allow_low_precision(reason="attention compute dtypes are "
                                       "deliberately reduced"), \
         tile.TileContext(nc) as tc:
        with tc.tile_pool(name="const", bufs=1) as cp, \
             tc.tile_pool(name="dram", bufs=1, space="DRAM") as dp:
            cos_sb = cp.tile([P, S], F32)
            sin_sb = cp.tile([P, S], F32)
            tri_sb = cp.tile([P, P], BF16)
            id_sb = cp.tile([P, P], BF16)
            oc_sb = cp.tile([P, 1], BF16)
            or_sb = cp.tile([1, P], F32R)
            nc.sync.dma_start(out=cos_sb, in_=cost[:, :])
            nc.sync.dma_start(out=sin_sb, in_=sints[:, :])
            nc.sync.dma_start(out=tri_sb, in_=trimask[:, :])
            nc.sync.dma_start(out=id_sb, in_=identb[:, :])
            nc.sync.dma_start(out=oc_sb, in_=onesc[:, :])
            nc.sync.dma_start(out=or_sb, in_=onesr[:, :])

            # per-batch scratch so phase 2 (batch 0) overlaps phase 1 (batch 1)
            q_scr = [dp.tile([FL, S], DT_QK, name=f"qscr{b}") for b in range(B)]
            k_scr = [dp.tile([DK, S], DT_QK, name=f"kscr{b}") for b in range(B)]
            v_scr = [dp.tile([S, DK], BF16, name=f"vscr{b}") for b in range(B)]

            # ---------------- Phase 1: QKV projection + RoPE ----------------
            with tc.tile_pool(name="wq", bufs=1) as wqp, \
                 tc.tile_pool(name="xp", bufs=3) as xp, \
                 tc.tile_pool(name="rp", bufs=2) as rp, \
                 tc.tile_pool(name="ps1", bufs=1, space="PSUM") as ps1, \
                 tc.tile_pool(name="pst", bufs=2, space="PSUM") as pst:
                wq_sb = wqp.tile([P, NA * FL], DT_PROJ)
                wk_sb = wqp.tile([P, NA * DK], DT_PROJ)
                wv_sb = wqp.tile([P, NA * DK], DT_PROJ)
                for a in range(NA):
                    nc.sync.dma_start(out=wq_sb[:, a * FL:(a + 1) * FL],
                                      in_=wqt[a * P:(a + 1) * P, :])
                    nc.sync.dma_start(out=wk_sb[:, a * DK:(a + 1) * DK],
                                      in_=wkt[a * P:(a + 1) * P, :])
                    nc.sync.dma_start(out=wv_sb[:, a * DK:(a + 1) * DK],
                                      in_=wvt[a * P:(a + 1) * P, :])

                for tb in range(NTB):
                    bi = (tb * TB) // S
                    s0 = (tb * TB) % S
                    psq = [ps1.tile([P, TB], F32, name=f"psq{j}_{tb}",
                                    tag=f"psq{j}") for j in range(NHL)]
                    psk = ps1.tile([P, TB], F32, name=f"psk_{tb}", tag="psk")
                    psv = ps1.tile([P, TB], F32, name=f"psv_{tb}", tag="psv")
                    for a in range(NA):
                        xt_t = xp.tile([P, TB], DT_PROJ, name=f"x_{tb}_{a}",
                                       tag="xt")
                        nc.sync.dma_start(
                            out=xt_t,
                            in_=xt[a * P:(a + 1) * P, tb * TB:(tb + 1) * TB])
                        st, sp = (a == 0), (a == NA - 1)
                        nc.tensor.matmul(psk, wk_sb[:, a * DK:(a + 1) * DK],
                                         xt_t, start=st, stop=sp)
                        nc.tensor.matmul(psv, wv_sb[:, a * DK:(a + 1) * DK],
                                         xt_t, start=st, stop=sp)
                        for j in range(NHL):
                            nc.tensor.matmul(
                                psq[j],
                                wq_sb[:, a * FL + j * DK:a * FL + (j + 1) * DK],
                                xt_t, start=st, stop=sp)

                    # Evict all 6 psum banks with a single ACT reader each so
                    # the next t-block's matmuls can reclaim banks at once.
                    evs = []
                    for src, scr, r0 in [(psq[0], q_scr, 0),
                                         (psq[1], q_scr, P),
                                         (psq[2], q_scr, 2 * P),
                                         (psq[3], q_scr, 3 * P),
                                         (psk, k_scr, 0)]:
                        qc = rp.tile([P, TB], F32, name=f"qc_{tb}_{r0}",
                                     tag=f"qc{r0}")
                        nc.scalar.copy(qc, src)
                        evs.append((qc, scr, r0))
                    vb = rp.tile([P, TB], BF16, name=f"vb_{tb}", tag="vb")
                    nc.scalar.copy(vb, psv)

                    # RoPE chains (read only SBUF now)
                    for qc, scr, r0 in evs:
                        sw = rp.tile([P, TB], F32, name=f"sw_{tb}_{r0}",
                                     tag=f"sw{r0}")
                        nc.sync.dma_start(out=sw[0:64, :], in_=qc[64:128, :])
                        nc.sync.dma_start(out=sw[64:128, :], in_=qc[0:64, :])
                        t1 = rp.tile([P, TB], F32, name=f"t1_{tb}_{r0}",
                                     tag=f"t1{r0}")
                        nc.vector.tensor_mul(t1, qc, cos_sb[:, s0:s0 + TB])
                        t2 = rp.tile([P, TB], F32, name=f"t2_{tb}_{r0}",
                                     tag=f"t2{r0}")
                        nc.vector.tensor_mul(t2, sw, sin_sb[:, s0:s0 + TB])
                        qf = rp.tile([P, TB], DT_QK, name=f"qf_{tb}_{r0}",
                                     tag=f"qf{r0}")
                        nc.vector.tensor_add(qf, t1, t2)
                        nc.sync.dma_start(
                            out=scr[bi][r0:r0 + P, s0:s0 + TB], in_=qf)

                    # V: transpose to token-major, store
                    for u in range(TB // P):
                        vt_ps = pst.tile([P, P], BF16, name=f"vt_{tb}_{u}",
                                         tag="vtp")
                        nc.tensor.transpose(vt_ps, vb[:, u * P:(u + 1) * P],
                                            id_sb)
                        vt_sb = rp.tile([P, P], BF16, name=f"vs_{tb}_{u}",
                                        tag="vts")
                        nc.vector.tensor_copy(vt_sb, vt_ps)
                        nc.sync.dma_start(
                            out=v_scr[bi][s0 + u * P:s0 + (u + 1) * P, :],
                            in_=vt_sb)

            # ---------------- Phase 2+3 pools ----------------
            with tc.tile_pool(name="attn", bufs=1) as ap:
                attn_sb = [[ap.tile([P, S], DT_ATT, name=f"attn{b}_{h}")
                            for h in range(NHL)] for b in range(B)]

                # ---------------- Phase 2: causal attention ----------------
                with tc.tile_pool(name="p2", bufs=1) as p2, \
                     tc.tile_pool(name="p2q", bufs=2) as p2q, \
                     tc.tile_pool(name="p2e", bufs=4) as p2e, \
                     tc.tile_pool(name="ps2s", bufs=2, space="PSUM") as ps2s, \
                     tc.tile_pool(name="ps2u", bufs=2, space="PSUM") as ps2u, \
                     tc.tile_pool(name="ps2r", bufs=2, space="PSUM") as ps2r:
                    for b in range(B):
                        kb_sb = p2q.tile([P, S], DT_QK, name=f"kb_{b}",
                                         tag="kb")
                        nc.sync.dma_start(out=kb_sb, in_=k_scr[b][:, :])
                        vtk = p2q.tile([P, NKT, P], BF16, name=f"vt_{b}",
                                       tag="vtk")
                        nc.sync.dma_start(
                            out=vtk,
                            in_=v_scr[b][:, :].rearrange("(n p) d -> p n d",
                                                         p=P))
                        for h in range(NHL):
                            qh_sb = p2q.tile([P, S], DT_QK, name=f"q_{b}_{h}",
                                             tag="qh")
                            nc.sync.dma_start(out=qh_sb,
                                              in_=q_scr[b][h * P:(h + 1) * P, :])

                            def make_norm(b, h, qb, u_ps, d_ps):
                                def norm():
                                    r_sb = p2.tile([1, QBS], F32R,
                                                   name=f"r_{b}_{h}_{qb}",
                                                   tag="r", bufs=2)
                                    nc.vector.reciprocal(r_sb, d_ps)
                                    rb_ps = ps2r.tile([P, QBS], F32,
                                                      name=f"rb_{b}_{h}_{qb}",
                                                      tag="rb")
                                    nc.tensor.matmul(rb_ps, or_sb, r_sb,
                                                     start=True, stop=True)
                                    rb_sb = p2.tile([P, QBS], F32,
                                                    name=f"rs_{b}_{h}_{qb}",
                                                    tag="rs", bufs=2)
                                    nc.scalar.copy(rb_sb, rb_ps)
                                    nc.vector.tensor_mul(
                                        attn_sb[b][h][:, qb * QBS:
                                                      (qb + 1) * QBS],
                                        u_ps, rb_sb)
                                return norm

                            pending = None
                            for qb in range(NQB):
                                nkt = 4 * qb + 4
                                u_ps = ps2u.tile([P, QBS], F32,
                                                 name=f"u_{b}_{h}_{qb}",
                                                 tag="u")
                                d_ps = ps2u.tile([1, QBS], F32,
                                                 name=f"d_{b}_{h}_{qb}",
                                                 tag="d")
                                for kt in range(nkt):
                                    s_ps = ps2s.tile(
                                        [P, QBS], F32,
                                        name=f"s_{b}_{h}_{qb}_{kt}", tag="s")
                                    nc.tensor.matmul(
                                        s_ps,
                                        kb_sb[:, kt * P:(kt + 1) * P],
                                        qh_sb[:, qb * QBS:(qb + 1) * QBS],
                                        start=True, stop=True)
                                    e_sb = p2e.tile(
                                        [P, QBS], BF16,
                                        name=f"e_{b}_{h}_{qb}_{kt}", tag="e")
                                    nc.scalar.activation(e_sb, s_ps, EXP,
                                                         scale=SCALE)
                                    m = kt - 4 * qb
                                    if m >= 0:
                                        if m > 0:
                                            nc.vector.memset(
                                                e_sb[:, 0:m * P], 0.0)
                                        nc.vector.tensor_mul(
                                            e_sb[:, m * P:(m + 1) * P],
                                            e_sb[:, m * P:(m + 1) * P],
                                            tri_sb)
                                    st, sp = (kt == 0), (kt == nkt - 1)
                                    nc.tensor.matmul(u_ps, vtk[:, kt, :],
                                                     e_sb, start=st, stop=sp)
                                    nc.tensor.matmul(d_ps, oc_sb, e_sb,
                                                     start=st, stop=sp)
                                    # emit the previous q-block's normalize
                                    # mid-loop so its PE broadcast never waits
                                    # on the DVE reciprocal
                                    if kt == 1 and pending is not None:
                                        pending()
                                        pending = None
                                pending = make_norm(b, h, qb, u_ps, d_ps)
                            pending()

                # ---------------- Phase 3: output projection ----------------
                with tc.tile_pool(name="p3w", bufs=2) as p3w, \
                     tc.tile_pool(name="p3o", bufs=4) as p3o, \
                     tc.tile_pool(name="ps3", bufs=4, space="PSUM") as ps3:
                    for ob in range(NOB):
                        wo_sb = p3w.tile([P, NHL, 512], DT_ATT,
                                         name=f"wo_{ob}", tag="wo")
                        for j in range(NHL):
                            nc.sync.dma_start(
                                out=wo_sb[:, j, :],
                                in_=wot[j * P:(j + 1) * P,
                                        ob * 512:(ob + 1) * 512])
                        for tt in range(NTT):
                            bt, st_ = tt // (S // P), (tt % (S // P)) * P
                            o_ps = ps3.tile([P, 512], F32,
                                            name=f"o_{ob}_{tt}", tag="o")
                            for j in range(NHL):
                                nc.tensor.matmul(
                                    o_ps,
                                    attn_sb[bt][j][:, st_:st_ + P],
                                    wo_sb[:, j, :],
                                    start=(j == 0), stop=(j == NHL - 1))
                            o_sb = p3o.tile([P, 512], F32,
                                            name=f"os_{ob}_{tt}", tag="os")
                            nc.scalar.copy(o_sb, o_ps)
                            nc.sync.dma_start(
                                out=out[tt * P:(tt + 1) * P,
                                        ob * 512:(ob + 1) * 512],
                                in_=o_sb)

    nc.compile()
    return nc


def _prep_inputs(hidden_states, Wq, Wk, Wv, Wo, cos, sin):
    hs = np.asarray(hidden_states, dtype=np.float32)
    Wq = np.asarray(Wq, dtype=np.float32)
    Wk = np.asarray(Wk, dtype=np.float32)
    Wv = np.asarray(Wv, dtype=np.float32)
    Wo = np.asarray(Wo, dtype=np.float32)
    cos = np.asarray(cos, dtype=np.float32)
    sin = np.asarray(sin, dtype=np.float32)

    xt = np.ascontiguousarray(hs.reshape(T, H).T).astype(NP_PROJ)
    cosT = np.ascontiguousarray(cos.T)
    sinT = np.ascontiguousarray(sin.T)
    sints = np.ascontiguousarray(
        np.concatenate([-sinT[:64], sinT[64:]], axis=0))
    kq = np.arange(P)
    trim = (kq[None, :] >= kq[:, None]).astype(ml_dtypes.bfloat16)
    ident = np.eye(P, dtype=ml_dtypes.bfloat16)
    onesc = np.ones((P, 1), dtype=ml_dtypes.bfloat16)
    onesr = np.ones((1, P), dtype=np.float32)

    in_maps = []
    for c in range(8):
        in_maps.append({
            "xt": xt,
            "wqt": np.ascontiguousarray(
                Wq[c * FL:(c + 1) * FL, :].T).astype(NP_PROJ),
            "wkt": np.ascontiguousarray(
                Wk[c * DK:(c + 1) * DK, :].T).astype(NP_PROJ),
            "wvt": np.ascontiguousarray(
                Wv[c * DK:(c + 1) * DK, :].T).astype(NP_PROJ),
            "wot": np.ascontiguousarray(
                Wo[:, c * FL:(c + 1) * FL].T).astype(NP_ATT),
            "cost": cosT,
            "sints": sints,
            "trimask": trim,
            "identb": ident,
            "onesc": onesc,
            "onesr": onesr,
        })
    return in_maps


def kernel(hidden_states, Wq, Wk, Wv, Wo, cos, sin, _run_kwargs=None):
    in_maps = _prep_inputs(hidden_states, Wq, Wk, Wv, Wo, cos, sin)
    if "nc" not in _NC_CACHE:
        _NC_CACHE["nc"] = build()
    nc = _NC_CACHE["nc"]
    kw = _run_kwargs or {}
    res = run_bass_kernel_spmd(nc, in_maps, core_ids=list(range(8)), **kw)
    acc = np.zeros((T, H), dtype=np.float64)
    for c in range(8):
        acc += np.asarray(res.results[c]["out"], dtype=np.float64)
    out = acc.astype(np.float32).reshape(B, S, H)
    if kw:
        _NC_CACHE["last_results"] = res
    return out


# revision 6
# speedup vs baseline: 1.1992x; 1.0976x over previous
"""Trainium2 Bass kernel for Llama GQA attention (B=2, S=2048, H=4096,
32 Q heads / 8 KV heads, head_dim 128, RoPE, causal).

Sharding: tensor-parallel by head across 8 cores. Core c owns Q heads
[4c..4c+3] and KV head c. Each core computes its Q/K/V projections,
RoPE, causal attention, and a partial output projection over its 512
attention features; the host sums the 8 partial outputs.

Device layout is feature-major ([feature, token]) throughout:
  - QKV proj:  Q'[f,t] (psum) = sum_h WqT[h,f].T @ xT[h,t]
  - RoPE:      q*cos + swap_halves(q)*sign*sin   (DVE + DMA swap)
  - scores:    S.T[k,q] = K'[d,k].T @ Q'[d,q]    (softmax over partition)
  - softmax:   exp on ACT (no max subtraction; scores are O(10)),
               denominator via ones-column matmul, reciprocal,
               broadcast via K=1 matmul, normalize on psum evict
  - AV:        U[d,q] = Vtok[k,d].T @ E[k,q]     (bf16)
  - out:       out[t,o] = attn'[f,t].T @ WoT[f,o]  (partial; host sums)
"""
import math
import numpy as np
import ml_dtypes

import concourse.bacc as bacc
import concourse.tile as tile
from concourse import mybir
from concourse.bass_utils import run_bass_kernel_spmd

F32 = mybir.dt.float32
F32R = mybir.dt.float32r
BF16 = mybir.dt.bfloat16

# Compute dtypes (bf16 matmuls stream at 1 cyc/row; f32r takes 2 passes)
DT_PROJ = BF16     # x / Wq / Wk / Wv and the QKV projection matmuls
DT_QK = BF16       # Q'/K' after rope -> scores matmul
DT_ATT = BF16      # attn' and WoT -> output projection matmul
NP_PROJ = ml_dtypes.bfloat16 if DT_PROJ == BF16 else np.float32
NP_ATT = ml_dtypes.bfloat16 if DT_ATT == BF16 else np.float32

P = 128
B, S, H = 2, 2048, 4096
T = B * S
DK = 128                     # head dim
NHL = 4                      # q heads per core
FL = NHL * DK                # 512 local q features
TB = 512                     # phase-1 token block
NTB = T // TB                # 8
NA = H // P                  # 32 contraction tiles
QBS = 512                    # attention q-block
NQB = S // QBS               # 4 q-blocks per (batch, head)
NKT = S // P                 # 16 k-tiles per batch
SCALE = 1.0 / math.sqrt(DK)
NOB = H // 512               # 8 output column blocks
NTT = T // P                 # 32 output row tiles

_NC_CACHE = {}


def build():
    nc = bacc.Bacc(None, target_bir_lowering=False)

    xt = nc.dram_tensor("xt", [H, T], DT_PROJ, kind="ExternalInput")
    wqt = nc.dram_tensor("wqt", [H, FL], DT_PROJ, kind="ExternalInput")
    wkt = nc.dram_tensor("wkt", [H, DK], DT_PROJ, kind="ExternalInput")
    wvt = nc.dram_tensor("wvt", [H, DK], DT_PROJ, kind="ExternalInput")
    wot = nc.dram_tensor("wot", [FL, H], DT_ATT, kind="ExternalInput")
    cost = nc.dram_tensor("cost", [P, S], F32, kind="ExternalInput")
    sints = nc.dram_tensor("sints", [P, S], F32, kind="ExternalInput")
    trimask = nc.dram_tensor("trimask", [P, P], BF16, kind="ExternalInput")
    identb = nc.dram_tensor("identb", [P, P], BF16, kind="ExternalInput")
    onesc = nc.dram_tensor("onesc", [P, 1], BF16, kind="ExternalInput")
    onesr = nc.dram_tensor("onesr", [1, P], F32R, kind="ExternalInput")
    out = nc.dram_tensor("out", [T, H], F32, kind="ExternalOutput")

    EXP = mybir.ActivationFunctionType.Exp

    with nc.allow_low_precision(reason="attention compute dtypes are "
                                       "deliberately reduced"), \
         tile.TileContext(nc) as tc:
        with tc.tile_pool(name="const", bufs=1) as cp, \
             tc.tile_pool(name="dram", bufs=1, space="DRAM") as dp:
            cos_sb = cp.tile([P, S], F32)
            sin_sb = cp.tile([P, S], F32)
            tri_sb = cp.tile([P, P], BF16)
            id_sb = cp.tile([P, P], BF16)
            oc_sb = cp.tile([P, 1], BF16)
            or_sb = cp.tile([1, P], F32R)
            nc.sync.dma_start(out=cos_sb, in_=cost[:, :])
            nc.sync.dma_start(out=sin_sb, in_=sints[:, :])
            nc.sync.dma_start(out=tri_sb, in_=trimask[:, :])
            nc.sync.dma_start(out=id_sb, in_=identb[:, :])
            nc.sync.dma_start(out=oc_sb, in_=onesc[:, :])
            nc.sync.dma_start(out=or_sb, in_=onesr[:, :])

            # per-batch scratch so phase 2 (batch 0) overlaps phase 1 (batch 1)
            q_scr = [dp.tile([FL, S], DT_QK, name=f"qscr{b}") for b in range(B)]
            k_scr = [dp.tile([DK, S], DT_QK, name=f"kscr{b}") for b in range(B)]
            v_scr = [dp.tile([S, DK], BF16, name=f"vscr{b}") for b in range(B)]

            # ---------------- Phase 1: QKV projection + RoPE ----------------
            with tc.tile_pool(name="wq", bufs=1) as wqp, \
                 tc.tile_pool(name="xp", bufs=3) as xp, \
                 tc.tile_pool(name="rp", bufs=2) as rp, \
                 tc.tile_pool(name="ps1", bufs=1, space="PSUM") as ps1, \
                 tc.tile_pool(name="pst", bufs=2, space="PSUM") as pst:
                wq_sb = wqp.tile([P, NA * FL], DT_PROJ)
                wk_sb = wqp.tile([P, NA * DK], DT_PROJ)
                wv_sb = wqp.tile([P, NA * DK], DT_PROJ)
                for a in range(NA):
                    nc.sync.dma_start(out=wq_sb[:, a * FL:(a + 1) * FL],
                                      in_=wqt[a * P:(a + 1) * P, :])
                    nc.sync.dma_start(out=wk_sb[:, a * DK:(a + 1) * DK],
                                      in_=wkt[a * P:(a + 1) * P, :])
                    nc.sync.dma_start(out=wv_sb[:, a * DK:(a + 1) * DK],
                                      in_=wvt[a * P:(a + 1) * P, :])

                pending_v = None
                for tb in range(NTB):
                    bi = (tb * TB) // S
                    s0 = (tb * TB) % S
                    psq = [ps1.tile([P, TB], F32, name=f"psq{j}_{tb}",
                                    tag=f"psq{j}") for j in range(NHL)]
                    psk = ps1.tile([P, TB], F32, name=f"psk_{tb}", tag="psk")
                    psv = ps1.tile([P, TB], F32, name=f"psv_{tb}", tag="psv")
                    for a in range(NA):
                        xt_t = xp.tile([P, TB], DT_PROJ, name=f"x_{tb}_{a}",
                                       tag="xt")
                        nc.sync.dma_start(
                            out=xt_t,
                            in_=xt[a * P:(a + 1) * P, tb * TB:(tb + 1) * TB])
                        st, sp = (a == 0), (a == NA - 1)
                        if a == 4 and pending_v is not None:
                            pending_v()
                            pending_v = None
                        nc.tensor.matmul(psk, wk_sb[:, a * DK:(a + 1) * DK],
                                         xt_t, start=st, stop=sp)
                        nc.tensor.matmul(psv, wv_sb[:, a * DK:(a + 1) * DK],
                                         xt_t, start=st, stop=sp)
                        for j in range(NHL):
                            nc.tensor.matmul(
                                psq[j],
                                wq_sb[:, a * FL + j * DK:a * FL + (j + 1) * DK],
                                xt_t, start=st, stop=sp)

                    # Evict all 6 psum banks with a single ACT reader each so
                    # the next t-block's matmuls can reclaim banks at once.
                    evs = []
                    for src, scr, r0 in [(psq[0], q_scr, 0),
                                         (psq[1], q_scr, P),
                                         (psq[2], q_scr, 2 * P),
                                         (psq[3], q_scr, 3 * P),
                                         (psk, k_scr, 0)]:
                        qc = rp.tile([P, TB], F32, name=f"qc_{tb}_{r0}",
                                     tag=f"qc{r0}")
                        nc.scalar.copy(qc, src)
                        evs.append((qc, scr, r0))
                    vb = rp.tile([P, TB], BF16, name=f"vb_{tb}", tag="vb")
                    nc.scalar.copy(vb, psv)

                    # RoPE chains (read only SBUF now)
                    for qc, scr, r0 in evs:
                        sw = rp.tile([P, TB], F32, name=f"sw_{tb}_{r0}",
                                     tag=f"sw{r0}")
                        nc.gpsimd.dma_start(out=sw[0:64, :], in_=qc[64:128, :])
                        nc.gpsimd.dma_start(out=sw[64:128, :], in_=qc[0:64, :])
                        t1 = rp.tile([P, TB], F32, name=f"t1_{tb}_{r0}",
                                     tag=f"t1{r0}")
                        nc.vector.tensor_mul(t1, qc, cos_sb[:, s0:s0 + TB])
                        t2 = rp.tile([P, TB], F32, name=f"t2_{tb}_{r0}",
                                     tag=f"t2{r0}")
                        nc.vector.tensor_mul(t2, sw, sin_sb[:, s0:s0 + TB])
                        qf = rp.tile([P, TB], DT_QK, name=f"qf_{tb}_{r0}",
                                     tag=f"qf{r0}")
                        nc.vector.tensor_add(qf, t1, t2)
                        nc.gpsimd.dma_start(
                            out=scr[bi][r0:r0 + P, s0:s0 + TB], in_=qf)

                    # V: transpose to token-major (deferred into the next
                    # t-block's matmul stream so PE never waits on the cast)
                    def make_vt(tb, bi, s0, vb):
                        def doit():
                            for u in range(TB // P):
                                vt_ps = pst.tile([P, P], BF16,
                                                 name=f"vt_{tb}_{u}",
                                                 tag="vtp")
                                nc.tensor.transpose(
                                    vt_ps, vb[:, u * P:(u + 1) * P], id_sb)
                                vt_sb = rp.tile([P, P], BF16,
                                                name=f"vs_{tb}_{u}",
                                                tag="vts")
                                nc.vector.tensor_copy(vt_sb, vt_ps)
                                nc.gpsimd.dma_start(
                                    out=v_scr[bi][s0 + u * P:
                                                  s0 + (u + 1) * P, :],
                                    in_=vt_sb)
                        return doit
                    pending_v = make_vt(tb, bi, s0, vb)
                if pending_v is not None:
                    pending_v()

            # ---------------- Phase 2+3 pools ----------------
            with tc.tile_pool(name="attn", bufs=1) as ap:
                attn_sb = [[ap.tile([P, S], DT_ATT, name=f"attn{b}_{h}")
                            for h in range(NHL)] for b in range(B)]

                # ---------------- Phase 2: causal attention ----------------
                with tc.tile_pool(name="p2", bufs=1) as p2, \
                     tc.tile_pool(name="p2q", bufs=2) as p2q, \
                     tc.tile_pool(name="p2e", bufs=4) as p2e, \
                     tc.tile_pool(name="ps2s", bufs=3, space="PSUM") as ps2s, \
                     tc.tile_pool(name="ps2u", bufs=2, space="PSUM") as ps2u, \
                     tc.tile_pool(name="ps2r", bufs=1, space="PSUM") as ps2r:
                    for b in range(B):
                        kb_sb = p2q.tile([P, S], DT_QK, name=f"kb_{b}",
                                         tag="kb")
                        nc.sync.dma_start(out=kb_sb, in_=k_scr[b][:, :])
                        vtk = p2q.tile([P, NKT, P], BF16, name=f"vt_{b}",
                                       tag="vtk")
                        nc.sync.dma_start(
                            out=vtk,
                            in_=v_scr[b][:, :].rearrange("(n p) d -> p n d",
                                                         p=P))
                        for h in range(NHL):
                            qh_sb = p2q.tile([P, S], DT_QK, name=f"q_{b}_{h}",
                                             tag="qh")
                            nc.sync.dma_start(out=qh_sb,
                                              in_=q_scr[b][h * P:(h + 1) * P, :])
                            if b == 0 and h == 0:
                                pending, mm_since = None, 0

                            def make_norm(b, h, qb, u_ps, d_ps):
                                def norm():
                                    r_sb = p2.tile([1, QBS], F32R,
                                                   name=f"r_{b}_{h}_{qb}",
                                                   tag="r", bufs=2)
                                    nc.vector.reciprocal(r_sb, d_ps)
                                    rb_ps = ps2r.tile([P, QBS], F32,
                                                      name=f"rb_{b}_{h}_{qb}",
                                                      tag="rb")
                                    nc.tensor.matmul(rb_ps, or_sb, r_sb,
                                                     start=True, stop=True)
                                    rb_sb = p2.tile([P, QBS], F32,
                                                    name=f"rs_{b}_{h}_{qb}",
                                                    tag="rs", bufs=2)
                                    nc.scalar.copy(rb_sb, rb_ps)
                                    nc.vector.tensor_mul(
                                        attn_sb[b][h][:, qb * QBS:
                                                      (qb + 1) * QBS],
                                        u_ps, rb_sb)
                                return norm

                            for qb in range(NQB):
                                nkt = 4 * qb + 4
                                u_ps = ps2u.tile([P, QBS], F32,
                                                 name=f"u_{b}_{h}_{qb}",
                                                 tag="u")
                                d_ps = ps2u.tile([1, QBS], F32,
                                                 name=f"d_{b}_{h}_{qb}",
                                                 tag="d")
                                for kt in range(nkt):
                                    s_ps = ps2s.tile(
                                        [P, QBS], F32,
                                        name=f"s_{b}_{h}_{qb}_{kt}", tag="s")
                                    nc.tensor.matmul(
                                        s_ps,
                                        kb_sb[:, kt * P:(kt + 1) * P],
                                        qh_sb[:, qb * QBS:(qb + 1) * QBS],
                                        start=True, stop=True)
                                    e_sb = p2e.tile(
                                        [P, QBS], BF16,
                                        name=f"e_{b}_{h}_{qb}_{kt}", tag="e")
                                    nc.scalar.activation(e_sb, s_ps, EXP,
                                                         scale=SCALE)
                                    m = kt - 4 * qb
                                    if m >= 0:
                                        if m > 0:
                                            nc.vector.memset(
                                                e_sb[:, 0:m * P], 0.0)
                                        nc.vector.tensor_mul(
                                            e_sb[:, m * P:(m + 1) * P],
                                            e_sb[:, m * P:(m + 1) * P],
                                            tri_sb)
                                    st, sp = (kt == 0), (kt == nkt - 1)
                                    nc.tensor.matmul(u_ps, vtk[:, kt, :],
                                                     e_sb, start=st, stop=sp)
                                    nc.tensor.matmul(d_ps, oc_sb, e_sb,
                                                     start=st, stop=sp)
                                    mm_since += 3
                                    # emit the previous q-block's normalize
                                    # once the PE has ~2.5us of queued work,
                                    # so its broadcast matmul never waits on
                                    # the DVE reciprocal
                                    if pending is not None and mm_since >= 12:
                                        pending()
                                        pending = None
                                pending = make_norm(b, h, qb, u_ps, d_ps)
                                mm_since = 0
                    pending()

                # ---------------- Phase 3: output projection ----------------
                with tc.tile_pool(name="p3w", bufs=2) as p3w, \
                     tc.tile_pool(name="p3o", bufs=4) as p3o, \
                     tc.tile_pool(name="ps3", bufs=4, space="PSUM") as ps3:
                    for ob in range(NOB):
                        wo_sb = p3w.tile([P, NHL, 512], DT_ATT,
                                         name=f"wo_{ob}", tag="wo")
                        for j in range(NHL):
                            nc.sync.dma_start(
                                out=wo_sb[:, j, :],
                                in_=wot[j * P:(j + 1) * P,
                                        ob * 512:(ob + 1) * 512])
                        for tt in range(NTT):
                            bt, st_ = tt // (S // P), (tt % (S // P)) * P
                            o_ps = ps3.tile([P, 512], F32,
                                            name=f"o_{ob}_{tt}", tag="o")
                            for j in range(NHL):
                                nc.tensor.matmul(
                                    o_ps,
                                    attn_sb[bt][j][:, st_:st_ + P],
                                    wo_sb[:, j, :],
                                    start=(j == 0), stop=(j == NHL - 1))
                            o_sb = p3o.tile([P, 512], F32,
                                            name=f"os_{ob}_{tt}", tag="os")
                            nc.scalar.copy(o_sb, o_ps)
                            nc.sync.dma_start(
                                out=out[tt * P:(tt + 1) * P,
                                        ob * 512:(ob + 1) * 512],
                                in_=o_sb)

    nc.compile()
    return nc


def _prep_inputs(hidden_states, Wq, Wk, Wv, Wo, cos, sin):
    hs = np.asarray(hidden_states, dtype=np.float32)
    Wq = np.asarray(Wq, dtype=np.float32)
    Wk = np.asarray(Wk, dtype=np.float32)
    Wv = np.asarray(Wv, dtype=np.float32)
    Wo = np.asarray(Wo, dtype=np.float32)
    cos = np.asarray(cos, dtype=np.float32)
    sin = np.asarray(sin, dtype=np.float32)

    xt = np.ascontiguousarray(hs.reshape(T, H).T).astype(NP_PROJ)
    cosT = np.ascontiguousarray(cos.T)
    sinT = np.ascontiguousarray(sin.T)
    sints = np.ascontiguousarray(
        np.concatenate([-sinT[:64], sinT[64:]], axis=0))
    kq = np.arange(P)
    trim = (kq[None, :] >= kq[:, None]).astype(ml_dtypes.bfloat16)
    ident = np.eye(P, dtype=ml_dtypes.bfloat16)
    onesc = np.ones((P, 1), dtype=ml_dtypes.bfloat16)
    onesr = np.ones((1, P), dtype=np.float32)

    in_maps = []
    for c in range(8):
        in_maps.append({
            "xt": xt,
            "wqt": np.ascontiguousarray(
                Wq[c * FL:(c + 1) * FL, :].T).astype(NP_PROJ),
            "wkt": np.ascontiguousarray(
                Wk[c * DK:(c + 1) * DK, :].T).astype(NP_PROJ),
            "wvt": np.ascontiguousarray(
                Wv[c * DK:(c + 1) * DK, :].T).astype(NP_PROJ),
            "wot": np.ascontiguousarray(
                Wo[:, c * FL:(c + 1) * FL].T).astype(NP_ATT),
            "cost": cosT,
            "sints": sints,
            "trimask": trim,
            "identb": ident,
            "onesc": onesc,
            "onesr": onesr,
        })
    return in_maps


def kernel(hidden_states, Wq, Wk, Wv, Wo, cos, sin, _run_kwargs=None):
    in_maps = _prep_inputs(hidden_states, Wq, Wk, Wv, Wo, cos, sin)
    if "nc" not in _NC_CACHE:
        _NC_CACHE["nc"] = build()
    nc = _NC_CACHE["nc"]
    kw = _run_kwargs or {}
    res = run_bass_kernel_spmd(nc, in_maps, core_ids=list(range(8)), **kw)
    acc = np.zeros((T, H), dtype=np.float64)
    for c in range(8):
        acc += np.asarray(res.results[c]["out"], dtype=np.float64)
    out = acc.astype(np.float32).reshape(B, S, H)
    if kw:
        _NC_CACHE["last_results"] = res
    return out


# revision 8
# speedup vs baseline: 1.2762x; 1.0642x over previous
"""Trainium2 Bass kernel for Llama GQA attention (B=2, S=2048, H=4096,
32 Q heads / 8 KV heads, head_dim 128, RoPE, causal).

Sharding: tensor-parallel by head across 8 cores. Core c owns Q heads
[4c..4c+3] and KV head c. Each core computes its Q/K/V projections,
RoPE, causal attention, and a partial output projection over its 512
attention features; the host sums the 8 partial outputs.

Device layout is feature-major ([feature, token]) throughout:
  - QKV proj:  Q'[f,t] (psum) = sum_h WqT[h,f].T @ xT[h,t]
  - RoPE:      q*cos + swap_halves(q)*sign*sin   (DVE + DMA swap)
  - scores:    S.T[k,q] = K'[d,k].T @ Q'[d,q]    (softmax over partition)
  - softmax:   exp on ACT (no max subtraction; scores are O(10)),
               denominator via ones-column matmul, reciprocal,
               broadcast via K=1 matmul, normalize on psum evict
  - AV:        U[d,q] = Vtok[k,d].T @ E[k,q]     (bf16)
  - out:       out[t,o] = attn'[f,t].T @ WoT[f,o]  (partial; host sums)
"""
import math
import numpy as np
import ml_dtypes

import concourse.bacc as bacc
import concourse.tile as tile
from concourse import mybir
from concourse.bass_utils import run_bass_kernel_spmd

F32 = mybir.dt.float32
F32R = mybir.dt.float32r
BF16 = mybir.dt.bfloat16

# Compute dtypes (bf16 matmuls stream at 1 cyc/row; f32r takes 2 passes)
DT_PROJ = BF16     # x / Wq / Wk / Wv and the QKV projection matmuls
DT_QK = BF16       # Q'/K' after rope -> scores matmul
DT_ATT = BF16      # attn' and WoT -> output projection matmul
NP_PROJ = ml_dtypes.bfloat16 if DT_PROJ == BF16 else np.float32
NP_ATT = ml_dtypes.bfloat16 if DT_ATT == BF16 else np.float32

P = 128
B, S, H = 2, 2048, 4096
T = B * S
DK = 128                     # head dim
NHL = 4                      # q heads per core
FL = NHL * DK                # 512 local q features
TB = 512                     # phase-1 token block
NTB = T // TB                # 8
NA = H // P                  # 32 contraction tiles
QBS = 512                    # attention q-block
NQB = S // QBS               # 4 q-blocks per (batch, head)
NKT = S // P                 # 16 k-tiles per batch
SCALE = 1.0 / math.sqrt(DK)
NOB = H // 512               # 8 output column blocks
NTT = T // P                 # 32 output row tiles

_NC_CACHE = {}


def build():
    nc = bacc.Bacc(None, target_bir_lowering=False)

    xt = nc.dram_tensor("xt", [H, T], DT_PROJ, kind="ExternalInput")
    wqt = nc.dram_tensor("wqt", [H, FL], DT_PROJ, kind="ExternalInput")
    wkt = nc.dram_tensor("wkt", [H, DK], DT_PROJ, kind="ExternalInput")
    wvt = nc.dram_tensor("wvt", [H, DK], DT_PROJ, kind="ExternalInput")
    wot = nc.dram_tensor("wot", [FL, H], DT_ATT, kind="ExternalInput")
    cost = nc.dram_tensor("cost", [P, S], F32, kind="ExternalInput")
    sints = nc.dram_tensor("sints", [P, S], F32, kind="ExternalInput")
    trimask = nc.dram_tensor("trimask", [P, P], BF16, kind="ExternalInput")
    identb = nc.dram_tensor("identb", [P, P], BF16, kind="ExternalInput")
    onesc = nc.dram_tensor("onesc", [P, 1], BF16, kind="ExternalInput")
    onesr = nc.dram_tensor("onesr", [1, P], F32R, kind="ExternalInput")
    out = nc.dram_tensor("out", [T, H], F32, kind="ExternalOutput")

    EXP = mybir.ActivationFunctionType.Exp

    with nc.allow_low_precision(reason="attention compute dtypes are "
                                       "deliberately reduced"), \
         tile.TileContext(nc) as tc:
        with tc.tile_pool(name="const", bufs=1) as cp, \
             tc.tile_pool(name="dram", bufs=1, space="DRAM") as dp:
            cos_sb = cp.tile([P, S], F32)
            sin_sb = cp.tile([P, S], F32)
            tri_sb = cp.tile([P, P], BF16)
            id_sb = cp.tile([P, P], BF16)
            oc_sb = cp.tile([P, 1], BF16)
            or_sb = cp.tile([1, P], F32R)
            nc.sync.dma_start(out=cos_sb, in_=cost[:, :])
            nc.sync.dma_start(out=sin_sb, in_=sints[:, :])
            nc.sync.dma_start(out=tri_sb, in_=trimask[:, :])
            nc.sync.dma_start(out=id_sb, in_=identb[:, :])
            nc.sync.dma_start(out=oc_sb, in_=onesc[:, :])
            nc.sync.dma_start(out=or_sb, in_=onesr[:, :])

            # per-batch scratch so phase 2 (batch 0) overlaps phase 1 (batch 1)
            q_scr = [dp.tile([FL, S], DT_QK, name=f"qscr{b}") for b in range(B)]
            k_scr = [dp.tile([DK, S], DT_QK, name=f"kscr{b}") for b in range(B)]
            v_scr = [dp.tile([S, DK], BF16, name=f"vscr{b}") for b in range(B)]

            # Long-lived phase-2 SBUF pools allocated BEFORE phase-1 pools
            # so their loads don't carry WAR deps on phase-1 weight reads.
            apctx = tc.tile_pool(name="attn", bufs=1)
            ap = apctx.__enter__()
            attn_sb = [[ap.tile([P, S], DT_ATT, name=f"attn{b}_{h}")
                        for h in range(NHL)] for b in range(B)]
            p2ctx = tc.tile_pool(name="p2", bufs=1)
            p2 = p2ctx.__enter__()
            p2qctx = tc.tile_pool(name="p2q", bufs=2)
            p2q = p2qctx.__enter__()
            p2ectx = tc.tile_pool(name="p2e", bufs=4)
            p2e = p2ectx.__enter__()

            # ---------------- Phase 1: QKV projection + RoPE ----------------
            with tc.tile_pool(name="wq", bufs=1) as wqp, \
                 tc.tile_pool(name="xp", bufs=3) as xp, \
                 tc.tile_pool(name="rp", bufs=1) as rp, \
                 tc.tile_pool(name="ps1", bufs=1, space="PSUM") as ps1, \
                 tc.tile_pool(name="pst", bufs=2, space="PSUM") as pst:
                wq_sb = wqp.tile([P, NA * FL], DT_PROJ)
                wk_sb = wqp.tile([P, NA * DK], DT_PROJ)
                wv_sb = wqp.tile([P, NA * DK], DT_PROJ)
                for a in range(NA):
                    nc.sync.dma_start(out=wq_sb[:, a * FL:(a + 1) * FL],
                                      in_=wqt[a * P:(a + 1) * P, :])
                    nc.sync.dma_start(out=wk_sb[:, a * DK:(a + 1) * DK],
                                      in_=wkt[a * P:(a + 1) * P, :])
                    nc.sync.dma_start(out=wv_sb[:, a * DK:(a + 1) * DK],
                                      in_=wvt[a * P:(a + 1) * P, :])

                pending_v = None
                for tb in range(NTB):
                    bi = (tb * TB) // S
                    s0 = (tb * TB) % S
                    psq = [ps1.tile([P, TB], F32, name=f"psq{j}_{tb}",
                                    tag=f"psq{j}") for j in range(NHL)]
                    psk = ps1.tile([P, TB], F32, name=f"psk_{tb}", tag="psk")
                    psv = ps1.tile([P, TB], F32, name=f"psv_{tb}", tag="psv")
                    for a in range(NA):
                        xt_t = xp.tile([P, TB], DT_PROJ, name=f"x_{tb}_{a}",
                                       tag="xt")
                        nc.sync.dma_start(
                            out=xt_t,
                            in_=xt[a * P:(a + 1) * P, tb * TB:(tb + 1) * TB])
                        st, sp = (a == 0), (a == NA - 1)
                        if a == 4 and pending_v is not None:
                            pending_v()
                            pending_v = None
                        nc.tensor.matmul(psk, wk_sb[:, a * DK:(a + 1) * DK],
                                         xt_t, start=st, stop=sp)
                        nc.tensor.matmul(psv, wv_sb[:, a * DK:(a + 1) * DK],
                                         xt_t, start=st, stop=sp)
                        for j in range(NHL):
                            nc.tensor.matmul(
                                psq[j],
                                wq_sb[:, a * FL + j * DK:a * FL + (j + 1) * DK],
                                xt_t, start=st, stop=sp)

                    # Evict all 6 psum banks with a single ACT reader each so
                    # the next t-block's matmuls can reclaim banks at once.
                    evs = []
                    for src, scr, r0 in [(psq[0], q_scr, 0),
                                         (psq[1], q_scr, P),
                                         (psq[2], q_scr, 2 * P),
                                         (psq[3], q_scr, 3 * P),
                                         (psk, k_scr, 0)]:
                        qc = rp.tile([P, TB], F32, name=f"qc_{tb}_{r0}",
                                     tag="qc", bufs=7)
                        nc.scalar.copy(qc, src)
                        evs.append((qc, scr, r0))
                    vb = rp.tile([P, TB], BF16, name=f"vb_{tb}", tag="vb",
                                 bufs=2)
                    nc.scalar.copy(vb, psv)

                    # RoPE chains (read only SBUF now; muls in place)
                    for qc, scr, r0 in evs:
                        sw = rp.tile([P, TB], F32, name=f"sw_{tb}_{r0}",
                                     tag="sw", bufs=7)
                        nc.gpsimd.dma_start(out=sw[0:64, :], in_=qc[64:128, :])
                        nc.gpsimd.dma_start(out=sw[64:128, :], in_=qc[0:64, :])
                        nc.vector.tensor_mul(qc, qc, cos_sb[:, s0:s0 + TB])
                        nc.vector.tensor_mul(sw, sw, sin_sb[:, s0:s0 + TB])
                        qf = rp.tile([P, TB], DT_QK, name=f"qf_{tb}_{r0}",
                                     tag="qf", bufs=7)
                        nc.vector.tensor_add(qf, qc, sw)
                        nc.gpsimd.dma_start(
                            out=scr[bi][r0:r0 + P, s0:s0 + TB], in_=qf)

                    # V: transpose to token-major (deferred into the next
                    # t-block's matmul stream so PE never waits on the cast)
                    def make_vt(tb, bi, s0, vb):
                        def doit():
                            for u in range(TB // P):
                                vt_ps = pst.tile([P, P], BF16,
                                                 name=f"vt_{tb}_{u}",
                                                 tag="vtp")
                                nc.tensor.transpose(
                                    vt_ps, vb[:, u * P:(u + 1) * P], id_sb)
                                vt_sb = rp.tile([P, P], BF16,
                                                name=f"vs_{tb}_{u}",
                                                tag="vts")
                                nc.vector.tensor_copy(vt_sb, vt_ps)
                                nc.gpsimd.dma_start(
                                    out=v_scr[bi][s0 + u * P:
                                                  s0 + (u + 1) * P, :],
                                    in_=vt_sb)
                        return doit
                    pending_v = make_vt(tb, bi, s0, vb)
                if pending_v is not None:
                    pending_v()

            # ---------------- Phase 2+3 ----------------
            if True:
                # ---------------- Phase 2: causal attention ----------------
                with tc.tile_pool(name="ps2s", bufs=3, space="PSUM") as ps2s, \
                     tc.tile_pool(name="ps2u", bufs=2, space="PSUM") as ps2u, \
                     tc.tile_pool(name="ps2r", bufs=1, space="PSUM") as ps2r:
                    for b in range(B):
                        kb_sb = p2q.tile([P, S], DT_QK, name=f"kb_{b}",
                                         tag="kb")
                        nc.sync.dma_start(out=kb_sb, in_=k_scr[b][:, :])
                        vtk = p2q.tile([P, NKT, P], BF16, name=f"vt_{b}",
                                       tag="vtk")
                        nc.sync.dma_start(
                            out=vtk,
                            in_=v_scr[b][:, :].rearrange("(n p) d -> p n d",
                                                         p=P))
                        for h in range(NHL):
                            qh_sb = p2q.tile([P, S], DT_QK, name=f"q_{b}_{h}",
                                             tag="qh")
                            nc.sync.dma_start(out=qh_sb,
                                              in_=q_scr[b][h * P:(h + 1) * P, :])
                            if b == 0 and h == 0:
                                pending, mm_since = None, 0

                            def make_norm(b, h, qb, u_ps, d_ps):
                                def norm():
                                    rf_sb = p2.tile([1, QBS], F32,
                                                    name=f"rf_{b}_{h}_{qb}",
                                                    tag="rf", bufs=2)
                                    nc.vector.reciprocal_approx_fast(rf_sb,
                                                                     d_ps)
                                    r_sb = p2.tile([1, QBS], F32R,
                                                   name=f"r_{b}_{h}_{qb}",
                                                   tag="r", bufs=2)
                                    nc.vector.tensor_copy(r_sb, rf_sb)
                                    rb_ps = ps2r.tile([P, QBS], F32,
                                                      name=f"rb_{b}_{h}_{qb}",
                                                      tag="rb")
                                    nc.tensor.matmul(rb_ps, or_sb, r_sb,
                                                     start=True, stop=True)
                                    rb_sb = p2.tile([P, QBS], F32,
                                                    name=f"rs_{b}_{h}_{qb}",
                                                    tag="rs", bufs=2)
                                    nc.scalar.copy(rb_sb, rb_ps)
                                    nc.vector.tensor_mul(
                                        attn_sb[b][h][:, qb * QBS:
                                                      (qb + 1) * QBS],
                                        u_ps, rb_sb)
                                return norm

                            for qb in range(NQB):
                                nkt = 4 * qb + 4
                                u_ps = ps2u.tile([P, QBS], F32,
                                                 name=f"u_{b}_{h}_{qb}",
                                                 tag="u")
                                d_ps = ps2u.tile([1, QBS], F32,
                                                 name=f"d_{b}_{h}_{qb}",
                                                 tag="d")
                                for kt in range(nkt):
                                    s_ps = ps2s.tile(
                                        [P, QBS], F32,
                                        name=f"s_{b}_{h}_{qb}_{kt}", tag="s")
                                    nc.tensor.matmul(
                                        s_ps,
                                        kb_sb[:, kt * P:(kt + 1) * P],
                                        qh_sb[:, qb * QBS:(qb + 1) * QBS],
                                        start=True, stop=True)
                                    e_sb = p2e.tile(
                                        [P, QBS], BF16,
                                        name=f"e_{b}_{h}_{qb}_{kt}", tag="e")
                                    nc.scalar.activation(e_sb, s_ps, EXP,
                                                         scale=SCALE)
                                    m = kt - 4 * qb
                                    if m >= 0:
                                        if m > 0:
                                            nc.vector.memset(
                                                e_sb[:, 0:m * P], 0.0)
                                        nc.vector.tensor_mul(
                                            e_sb[:, m * P:(m + 1) * P],
                                            e_sb[:, m * P:(m + 1) * P],
                                            tri_sb)
                                    st, sp = (kt == 0), (kt == nkt - 1)
                                    nc.tensor.matmul(u_ps, vtk[:, kt, :],
                                                     e_sb, start=st, stop=sp)
                                    nc.tensor.matmul(d_ps, oc_sb, e_sb,
                                                     start=st, stop=sp)
                                    mm_since += 3
                                    # emit the previous q-block's normalize
                                    # once the PE has ~2.5us of queued work,
                                    # so its broadcast matmul never waits on
                                    # the DVE reciprocal
                                    if pending is not None and mm_since >= 12:
                                        pending()
                                        pending = None
                                pending = make_norm(b, h, qb, u_ps, d_ps)
                                mm_since = 0
                    pending()

                p2ectx.__exit__(None, None, None)
                p2qctx.__exit__(None, None, None)
                p2ctx.__exit__(None, None, None)

                # ---------------- Phase 3: output projection ----------------
                with tc.tile_pool(name="p3w", bufs=2) as p3w, \
                     tc.tile_pool(name="p3o", bufs=4) as p3o, \
                     tc.tile_pool(name="ps3", bufs=4, space="PSUM") as ps3:
                    for ob in range(NOB):
                        wo_sb = p3w.tile([P, NHL, 512], DT_ATT,
                                         name=f"wo_{ob}", tag="wo")
                        for j in range(NHL):
                            nc.sync.dma_start(
                                out=wo_sb[:, j, :],
                                in_=wot[j * P:(j + 1) * P,
                                        ob * 512:(ob + 1) * 512])
                        for tt in range(NTT):
                            bt, st_ = tt // (S // P), (tt % (S // P)) * P
                            o_ps = ps3.tile([P, 512], F32,
                                            name=f"o_{ob}_{tt}", tag="o")
                            for j in range(NHL):
                                nc.tensor.matmul(
                                    o_ps,
                                    attn_sb[bt][j][:, st_:st_ + P],
                                    wo_sb[:, j, :],
                                    start=(j == 0), stop=(j == NHL - 1))
                            o_sb = p3o.tile([P, 512], F32,
                                            name=f"os_{ob}_{tt}", tag="os")
                            nc.vector.tensor_copy(o_sb, o_ps)
                            nc.sync.dma_start(
                                out=out[tt * P:(tt + 1) * P,
                                        ob * 512:(ob + 1) * 512],
                                in_=o_sb)
                apctx.__exit__(None, None, None)

    nc.compile()
    return nc


def _prep_inputs(hidden_states, Wq, Wk, Wv, Wo, cos, sin):
    hs = np.asarray(hidden_states, dtype=np.float32)
    Wq = np.asarray(Wq, dtype=np.float32)
    Wk = np.asarray(Wk, dtype=np.float32)
    Wv = np.asarray(Wv, dtype=np.float32)
    Wo = np.asarray(Wo, dtype=np.float32)
    cos = np.asarray(cos, dtype=np.float32)
    sin = np.asarray(sin, dtype=np.float32)

    xt = np.ascontiguousarray(hs.reshape(T, H).T).astype(NP_PROJ)
    cosT = np.ascontiguousarray(cos.T)
    sinT = np.ascontiguousarray(sin.T)
    sints = np.ascontiguousarray(
        np.concatenate([-sinT[:64], sinT[64:]], axis=0))
    kq = np.arange(P)
    trim = (kq[None, :] >= kq[:, None]).astype(ml_dtypes.bfloat16)
    ident = np.eye(P, dtype=ml_dtypes.bfloat16)
    onesc = np.ones((P, 1), dtype=ml_dtypes.bfloat16)
    onesr = np.ones((1, P), dtype=np.float32)

    in_maps = []
    for c in range(8):
        in_maps.append({
            "xt": xt,
            "wqt": np.ascontiguousarray(
                Wq[c * FL:(c + 1) * FL, :].T).astype(NP_PROJ),
            "wkt": np.ascontiguousarray(
                Wk[c * DK:(c + 1) * DK, :].T).astype(NP_PROJ),
            "wvt": np.ascontiguousarray(
                Wv[c * DK:(c + 1) * DK, :].T).astype(NP_PROJ),
            "wot": np.ascontiguousarray(
                Wo[:, c * FL:(c + 1) * FL].T).astype(NP_ATT),
            "cost": cosT,
            "sints": sints,
            "trimask": trim,
            "identb": ident,
            "onesc": onesc,
            "onesr": onesr,
        })
    return in_maps


def kernel(hidden_states, Wq, Wk, Wv, Wo, cos, sin, _run_kwargs=None):
    in_maps = _prep_inputs(hidden_states, Wq, Wk, Wv, Wo, cos, sin)
    if "nc" not in _NC_CACHE:
        _NC_CACHE["nc"] = build()
    nc = _NC_CACHE["nc"]
    kw = _run_kwargs or {}
    res = run_bass_kernel_spmd(nc, in_maps, core_ids=list(range(8)), **kw)
    acc = np.zeros((T, H), dtype=np.float64)
    for c in range(8):
        acc += np.asarray(res.results[c]["out"], dtype=np.float64)
    out = acc.astype(np.float32).reshape(B, S, H)
    if kw:
        _NC_CACHE["last_results"] = res
    return out


# revision 9
# speedup vs baseline: 1.3243x; 1.0377x over previous
"""Trainium2 Bass kernel for Llama GQA attention (B=2, S=2048, H=4096,
32 Q heads / 8 KV heads, head_dim 128, RoPE, causal).

Sharding: tensor-parallel by head across 8 cores. Core c owns Q heads
[4c..4c+3] and KV head c. Each core computes its Q/K/V projections,
RoPE, causal attention, and a partial output projection over its 512
attention features; the host sums the 8 partial outputs.

Device layout is feature-major ([feature, token]) throughout:
  - QKV proj:  Q'[f,t] (psum) = sum_h WqT[h,f].T @ xT[h,t]
  - RoPE:      q*cos + swap_halves(q)*sign*sin   (DVE + DMA swap)
  - scores:    S.T[k,q] = K'[d,k].T @ Q'[d,q]    (softmax over partition)
  - softmax:   exp on ACT (no max subtraction; scores are O(10)),
               denominator via ones-column matmul, reciprocal,
               broadcast via K=1 matmul, normalize on psum evict
  - AV:        U[d,q] = Vtok[k,d].T @ E[k,q]     (bf16)
  - out:       out[t,o] = attn'[f,t].T @ WoT[f,o]  (partial; host sums)
"""
import math
import numpy as np
import ml_dtypes

import concourse.bacc as bacc
import concourse.tile as tile
from concourse import mybir
from concourse.bass_utils import run_bass_kernel_spmd

F32 = mybir.dt.float32
F32R = mybir.dt.float32r
BF16 = mybir.dt.bfloat16

# Compute dtypes (bf16 matmuls stream at 1 cyc/row; f32r takes 2 passes)
DT_PROJ = BF16     # x / Wq / Wk / Wv and the QKV projection matmuls
DT_QK = BF16       # Q'/K' after rope -> scores matmul
DT_ATT = BF16      # attn' and WoT -> output projection matmul
NP_PROJ = ml_dtypes.bfloat16 if DT_PROJ == BF16 else np.float32
NP_ATT = ml_dtypes.bfloat16 if DT_ATT == BF16 else np.float32

P = 128
B, S, H = 2, 2048, 4096
T = B * S
DK = 128                     # head dim
NHL = 4                      # q heads per core
FL = NHL * DK                # 512 local q features
TB = 512                     # phase-1 token block
NTB = T // TB                # 8
NA = H // P                  # 32 contraction tiles
QBS = 512                    # attention q-block
NQB = S // QBS               # 4 q-blocks per (batch, head)
NKT = S // P                 # 16 k-tiles per batch
SCALE = 1.0 / math.sqrt(DK)
NOB = H // 512               # 8 output column blocks
NTT = T // P                 # 32 output row tiles

_NC_CACHE = {}


def build():
    nc = bacc.Bacc(None, target_bir_lowering=False)

    xt = nc.dram_tensor("xt", [H, T], DT_PROJ, kind="ExternalInput")
    wqt = nc.dram_tensor("wqt", [H, FL], DT_PROJ, kind="ExternalInput")
    wkt = nc.dram_tensor("wkt", [H, DK], DT_PROJ, kind="ExternalInput")
    wvt = nc.dram_tensor("wvt", [H, DK], DT_PROJ, kind="ExternalInput")
    wot = nc.dram_tensor("wot", [FL, H], DT_ATT, kind="ExternalInput")
    cost = nc.dram_tensor("cost", [P, S], F32, kind="ExternalInput")
    sints = nc.dram_tensor("sints", [P, S], F32, kind="ExternalInput")
    trimask = nc.dram_tensor("trimask", [P, P], BF16, kind="ExternalInput")
    identb = nc.dram_tensor("identb", [P, P], BF16, kind="ExternalInput")
    onesc = nc.dram_tensor("onesc", [P, 1], BF16, kind="ExternalInput")
    onesr = nc.dram_tensor("onesr", [1, P], F32R, kind="ExternalInput")
    out = nc.dram_tensor("out", [T, H], F32, kind="ExternalOutput")

    EXP = mybir.ActivationFunctionType.Exp

    with nc.allow_low_precision(reason="attention compute dtypes are "
                                       "deliberately reduced"), \
         tile.TileContext(nc) as tc:
        with tc.tile_pool(name="const", bufs=1) as cp, \
             tc.tile_pool(name="dram", bufs=1, space="DRAM") as dp:
            cos_sb = cp.tile([P, S], F32)
            sin_sb = cp.tile([P, S], F32)
            tri_sb = cp.tile([P, P], BF16)
            id_sb = cp.tile([P, P], BF16)
            oc_sb = cp.tile([P, 1], BF16)
            or_sb = cp.tile([1, P], F32R)
            nc.sync.dma_start(out=cos_sb, in_=cost[:, :])
            nc.sync.dma_start(out=sin_sb, in_=sints[:, :])
            nc.sync.dma_start(out=tri_sb, in_=trimask[:, :])
            nc.sync.dma_start(out=id_sb, in_=identb[:, :])
            nc.sync.dma_start(out=oc_sb, in_=onesc[:, :])
            nc.sync.dma_start(out=or_sb, in_=onesr[:, :])

            # per-batch scratch so phase 2 (batch 0) overlaps phase 1 (batch 1)
            q_scr = [dp.tile([FL, S], DT_QK, name=f"qscr{b}") for b in range(B)]
            k_scr = [dp.tile([DK, S], DT_QK, name=f"kscr{b}") for b in range(B)]
            v_scr = [dp.tile([S, DK], BF16, name=f"vscr{b}") for b in range(B)]

            # Long-lived phase-2 SBUF pools allocated BEFORE phase-1 pools
            # so their loads don't carry WAR deps on phase-1 weight reads.
            apctx = tc.tile_pool(name="attn", bufs=1)
            ap = apctx.__enter__()
            attn_sb = [[ap.tile([P, S], DT_ATT, name=f"attn{b}_{h}")
                        for h in range(NHL)] for b in range(B)]
            p2ctx = tc.tile_pool(name="p2", bufs=1)
            p2 = p2ctx.__enter__()
            p2qctx = tc.tile_pool(name="p2q", bufs=2)
            p2q = p2qctx.__enter__()
            p2ectx = tc.tile_pool(name="p2e", bufs=4)
            p2e = p2ectx.__enter__()

            # ---------------- Phase 1: QKV projection + RoPE ----------------
            with tc.tile_pool(name="wq", bufs=1) as wqp, \
                 tc.tile_pool(name="xp", bufs=3) as xp, \
                 tc.tile_pool(name="rp", bufs=1) as rp, \
                 tc.tile_pool(name="ps1", bufs=1, space="PSUM") as ps1, \
                 tc.tile_pool(name="pst", bufs=2, space="PSUM") as pst:
                wq_sb = wqp.tile([P, NA * FL], DT_PROJ)
                wk_sb = wqp.tile([P, NA * DK], DT_PROJ)
                wv_sb = wqp.tile([P, NA * DK], DT_PROJ)
                for a in range(NA):
                    nc.sync.dma_start(out=wq_sb[:, a * FL:(a + 1) * FL],
                                      in_=wqt[a * P:(a + 1) * P, :])
                    nc.sync.dma_start(out=wk_sb[:, a * DK:(a + 1) * DK],
                                      in_=wkt[a * P:(a + 1) * P, :])
                    nc.sync.dma_start(out=wv_sb[:, a * DK:(a + 1) * DK],
                                      in_=wvt[a * P:(a + 1) * P, :])

                pending_v = None
                for tb in range(NTB):
                    bi = (tb * TB) // S
                    s0 = (tb * TB) % S
                    psq = [ps1.tile([P, TB], F32, name=f"psq{j}_{tb}",
                                    tag=f"psq{j}") for j in range(NHL)]
                    psk = ps1.tile([P, TB], F32, name=f"psk_{tb}", tag="psk")
                    psv = ps1.tile([P, TB], F32, name=f"psv_{tb}", tag="psv")
                    for a in range(NA):
                        xt_t = xp.tile([P, TB], DT_PROJ, name=f"x_{tb}_{a}",
                                       tag="xt")
                        nc.sync.dma_start(
                            out=xt_t,
                            in_=xt[a * P:(a + 1) * P, tb * TB:(tb + 1) * TB])
                        st, sp = (a == 0), (a == NA - 1)
                        if a == 4 and pending_v is not None:
                            pending_v()
                            pending_v = None
                        nc.tensor.matmul(psk, wk_sb[:, a * DK:(a + 1) * DK],
                                         xt_t, start=st, stop=sp)
                        nc.tensor.matmul(psv, wv_sb[:, a * DK:(a + 1) * DK],
                                         xt_t, start=st, stop=sp)
                        for j in range(NHL):
                            nc.tensor.matmul(
                                psq[j],
                                wq_sb[:, a * FL + j * DK:a * FL + (j + 1) * DK],
                                xt_t, start=st, stop=sp)

                    # Evict all 6 psum banks with a single ACT reader each so
                    # the next t-block's matmuls can reclaim banks at once.
                    evs = []
                    plan = [(psk, k_scr, 0, nc.scalar),
                            (psq[0], q_scr, 0, nc.vector),
                            (psq[1], q_scr, P, nc.scalar),
                            (psq[2], q_scr, 2 * P, nc.vector),
                            (psq[3], q_scr, 3 * P, nc.scalar)]
                    for idx, (src, scr, r0, eng) in enumerate(plan):
                        qc = rp.tile([P, TB], F32, name=f"qc_{tb}_{idx}",
                                     tag="qc", bufs=7)
                        if eng is nc.scalar:
                            nc.scalar.copy(qc, src)
                        else:
                            nc.vector.tensor_copy(qc, src)
                        if idx == 0:
                            vb = rp.tile([P, TB], BF16, name=f"vb_{tb}",
                                         tag="vb", bufs=2)
                            nc.vector.tensor_copy(vb, psv)
                        evs.append((qc, scr, r0))

                    # RoPE chains (read only SBUF now; muls in place)
                    for qc, scr, r0 in evs:
                        sw = rp.tile([P, TB], F32, name=f"sw_{tb}_{r0}",
                                     tag="sw", bufs=7)
                        nc.gpsimd.dma_start(out=sw[0:64, :], in_=qc[64:128, :])
                        nc.gpsimd.dma_start(out=sw[64:128, :], in_=qc[0:64, :])
                        nc.vector.tensor_mul(qc, qc, cos_sb[:, s0:s0 + TB])
                        nc.vector.tensor_mul(sw, sw, sin_sb[:, s0:s0 + TB])
                        qf = rp.tile([P, TB], DT_QK, name=f"qf_{tb}_{r0}",
                                     tag="qf", bufs=7)
                        nc.vector.tensor_add(qf, qc, sw)
                        nc.gpsimd.dma_start(
                            out=scr[bi][r0:r0 + P, s0:s0 + TB], in_=qf)

                    # V: transpose to token-major (deferred into the next
                    # t-block's matmul stream so PE never waits on the cast)
                    def make_vt(tb, bi, s0, vb):
                        def doit():
                            for u in range(TB // P):
                                vt_ps = pst.tile([P, P], BF16,
                                                 name=f"vt_{tb}_{u}",
                                                 tag="vtp")
                                nc.tensor.transpose(
                                    vt_ps, vb[:, u * P:(u + 1) * P], id_sb)
                                vt_sb = rp.tile([P, P], BF16,
                                                name=f"vs_{tb}_{u}",
                                                tag="vts")
                                nc.vector.tensor_copy(vt_sb, vt_ps)
                                nc.gpsimd.dma_start(
                                    out=v_scr[bi][s0 + u * P:
                                                  s0 + (u + 1) * P, :],
                                    in_=vt_sb)
                        return doit
                    pending_v = make_vt(tb, bi, s0, vb)
                if pending_v is not None:
                    pending_v()

            # ---------------- Phase 2+3 ----------------
            if True:
                # ---------------- Phase 2: causal attention ----------------
                with tc.tile_pool(name="ps2s", bufs=3, space="PSUM") as ps2s, \
                     tc.tile_pool(name="ps2u", bufs=2, space="PSUM") as ps2u, \
                     tc.tile_pool(name="ps2r", bufs=1, space="PSUM") as ps2r:
                    for b in range(B):
                        kb_sb = p2q.tile([P, S], DT_QK, name=f"kb_{b}",
                                         tag="kb")
                        nc.sync.dma_start(out=kb_sb, in_=k_scr[b][:, :])
                        vtk = p2q.tile([P, NKT, P], BF16, name=f"vt_{b}",
                                       tag="vtk")
                        nc.sync.dma_start(
                            out=vtk,
                            in_=v_scr[b][:, :].rearrange("(n p) d -> p n d",
                                                         p=P))
                        for h in range(NHL):
                            qh_sb = p2q.tile([P, S], DT_QK, name=f"q_{b}_{h}",
                                             tag="qh")
                            nc.sync.dma_start(out=qh_sb,
                                              in_=q_scr[b][h * P:(h + 1) * P, :])
                            if b == 0 and h == 0:
                                pending, mm_since = None, 0

                            def make_norm(b, h, qb, u_ps, d_ps):
                                def norm():
                                    rf_sb = p2.tile([1, QBS], F32,
                                                    name=f"rf_{b}_{h}_{qb}",
                                                    tag="rf", bufs=2)
                                    nc.vector.reciprocal_approx_fast(rf_sb,
                                                                     d_ps)
                                    r_sb = p2.tile([1, QBS], F32R,
                                                   name=f"r_{b}_{h}_{qb}",
                                                   tag="r", bufs=2)
                                    nc.vector.tensor_copy(r_sb, rf_sb)
                                    rb_ps = ps2r.tile([P, QBS], F32,
                                                      name=f"rb_{b}_{h}_{qb}",
                                                      tag="rb")
                                    nc.tensor.matmul(rb_ps, or_sb, r_sb,
                                                     start=True, stop=True)
                                    rb_sb = p2.tile([P, QBS], F32,
                                                    name=f"rs_{b}_{h}_{qb}",
                                                    tag="rs", bufs=2)
                                    nc.scalar.copy(rb_sb, rb_ps)
                                    nc.vector.tensor_mul(
                                        attn_sb[b][h][:, qb * QBS:
                                                      (qb + 1) * QBS],
                                        u_ps, rb_sb)
                                return norm

                            for qb in range(NQB):
                                nkt = 4 * qb + 4
                                u_ps = ps2u.tile([P, QBS], F32,
                                                 name=f"u_{b}_{h}_{qb}",
                                                 tag="u")
                                d_ps = ps2u.tile([1, QBS], F32,
                                                 name=f"d_{b}_{h}_{qb}",
                                                 tag="d")
                                def emit_av(kt, e_sb, lo):
                                    st, sp = (kt == 0), (kt == nkt - 1)
                                    nc.tensor.matmul(u_ps[:, lo:], vtk[:, kt, :],
                                                     e_sb[:, lo:],
                                                     start=st, stop=sp,
                                                     skip_group_check=True)
                                    nc.tensor.matmul(d_ps[:, lo:], oc_sb,
                                                     e_sb[:, lo:],
                                                     start=st, stop=sp,
                                                     skip_group_check=True)
                                prev_av = None
                                for kt in range(nkt):
                                    s_ps = ps2s.tile(
                                        [P, QBS], F32,
                                        name=f"s_{b}_{h}_{qb}_{kt}", tag="s")
                                    m = kt - 4 * qb
                                    lo = m * P if m > 0 else 0
                                    nc.tensor.matmul(
                                        s_ps[:, lo:],
                                        kb_sb[:, kt * P:(kt + 1) * P],
                                        qh_sb[:, qb * QBS + lo:
                                              (qb + 1) * QBS],
                                        start=True, stop=True)
                                    e_sb = p2e.tile(
                                        [P, QBS], BF16,
                                        name=f"e_{b}_{h}_{qb}_{kt}", tag="e")
                                    nc.scalar.activation(e_sb[:, lo:],
                                                         s_ps[:, lo:], EXP,
                                                         scale=SCALE)
                                    if m >= 0:
                                        nc.vector.tensor_mul(
                                            e_sb[:, m * P:(m + 1) * P],
                                            e_sb[:, m * P:(m + 1) * P],
                                            tri_sb)
                                    if prev_av is not None:
                                        emit_av(*prev_av)
                                    prev_av = (kt, e_sb, lo)
                                    mm_since += 3
                                    if pending is not None and mm_since >= 12:
                                        pending()
                                        pending = None
                                emit_av(*prev_av)
                                pending = make_norm(b, h, qb, u_ps, d_ps)
                                mm_since = 0
                    pending()

                p2ectx.__exit__(None, None, None)
                p2qctx.__exit__(None, None, None)
                p2ctx.__exit__(None, None, None)

                # ---------------- Phase 3: output projection ----------------
                with tc.tile_pool(name="p3w", bufs=2) as p3w, \
                     tc.tile_pool(name="p3o", bufs=4) as p3o, \
                     tc.tile_pool(name="ps3", bufs=4, space="PSUM") as ps3:
                    wo_tiles = {}

                    def load_wo(ob):
                        wo_sb = p3w.tile([P, NHL, 512], DT_ATT,
                                         name=f"wo_{ob}", tag="wo")
                        for j in range(NHL):
                            nc.sync.dma_start(
                                out=wo_sb[:, j, :],
                                in_=wot[j * P:(j + 1) * P,
                                        ob * 512:(ob + 1) * 512])
                        wo_tiles[ob] = wo_sb

                    load_wo(0)
                    for ob in range(NOB):
                        wo_sb = wo_tiles.pop(ob)
                        for tt in range(NTT):
                            if tt == 4 and ob + 1 < NOB:
                                load_wo(ob + 1)
                            bt, st_ = tt // (S // P), (tt % (S // P)) * P
                            o_ps = ps3.tile([P, 512], F32,
                                            name=f"o_{ob}_{tt}", tag="o")
                            for j in range(NHL):
                                nc.tensor.matmul(
                                    o_ps,
                                    attn_sb[bt][j][:, st_:st_ + P],
                                    wo_sb[:, j, :],
                                    start=(j == 0), stop=(j == NHL - 1))
                            o_sb = p3o.tile([P, 512], F32,
                                            name=f"os_{ob}_{tt}", tag="os")
                            nc.vector.tensor_copy(o_sb, o_ps)
                            nc.sync.dma_start(
                                out=out[tt * P:(tt + 1) * P,
                                        ob * 512:(ob + 1) * 512],
                                in_=o_sb)
                apctx.__exit__(None, None, None)

    nc.compile()
    return nc


def _prep_inputs(hidden_states, Wq, Wk, Wv, Wo, cos, sin):
    hs = np.asarray(hidden_states, dtype=np.float32)
    Wq = np.asarray(Wq, dtype=np.float32)
    Wk = np.asarray(Wk, dtype=np.float32)
    Wv = np.asarray(Wv, dtype=np.float32)
    Wo = np.asarray(Wo, dtype=np.float32)
    cos = np.asarray(cos, dtype=np.float32)
    sin = np.asarray(sin, dtype=np.float32)

    xt = np.ascontiguousarray(hs.reshape(T, H).T).astype(NP_PROJ)
    cosT = np.ascontiguousarray(cos.T)
    sinT = np.ascontiguousarray(sin.T)
    sints = np.ascontiguousarray(
        np.concatenate([-sinT[:64], sinT[64:]], axis=0))
    kq = np.arange(P)
    trim = (kq[None, :] >= kq[:, None]).astype(ml_dtypes.bfloat16)
    ident = np.eye(P, dtype=ml_dtypes.bfloat16)
    onesc = np.ones((P, 1), dtype=ml_dtypes.bfloat16)
    onesr = np.ones((1, P), dtype=np.float32)

    in_maps = []
    for c in range(8):
        in_maps.append({
            "xt": xt,
            "wqt": np.ascontiguousarray(
                Wq[c * FL:(c + 1) * FL, :].T).astype(NP_PROJ),
            "wkt": np.ascontiguousarray(
                Wk[c * DK:(c + 1) * DK, :].T).astype(NP_PROJ),
            "wvt": np.ascontiguousarray(
                Wv[c * DK:(c + 1) * DK, :].T).astype(NP_PROJ),
            "wot": np.ascontiguousarray(
                Wo[:, c * FL:(c + 1) * FL].T).astype(NP_ATT),
            "cost": cosT,
            "sints": sints,
            "trimask": trim,
            "identb": ident,
            "onesc": onesc,
            "onesr": onesr,
        })
    return in_maps


def kernel(hidden_states, Wq, Wk, Wv, Wo, cos, sin, _run_kwargs=None):
    in_maps = _prep_inputs(hidden_states, Wq, Wk, Wv, Wo, cos, sin)
    if "nc" not in _NC_CACHE:
        _NC_CACHE["nc"] = build()
    nc = _NC_CACHE["nc"]
    kw = _run_kwargs or {}
    res = run_bass_kernel_spmd(nc, in_maps, core_ids=list(range(8)), **kw)
    acc = np.zeros((T, H), dtype=np.float64)
    for c in range(8):
        acc += np.asarray(res.results[c]["out"], dtype=np.float64)
    out = acc.astype(np.float32).reshape(B, S, H)
    if kw:
        _NC_CACHE["last_results"] = res
    return out


# revision 10
# speedup vs baseline: 1.4095x; 1.0643x over previous
"""Trainium2 Bass kernel for Llama GQA attention (B=2, S=2048, H=4096,
32 Q heads / 8 KV heads, head_dim 128, RoPE, causal).

Sharding: tensor-parallel by head across 8 cores. Core c owns Q heads
[4c..4c+3] and KV head c. Each core computes its Q/K/V projections,
RoPE, causal attention, and a partial output projection over its 512
attention features; the host sums the 8 partial outputs.

Device layout is feature-major ([feature, token]) throughout:
  - QKV proj:  Q'[f,t] (psum) = sum_h WqT[h,f].T @ xT[h,t]
  - RoPE:      q*cos + swap_halves(q)*sign*sin   (DVE + DMA swap)
  - scores:    S.T[k,q] = K'[d,k].T @ Q'[d,q]    (softmax over partition)
  - softmax:   exp on ACT (no max subtraction; scores are O(10)),
               denominator via ones-column matmul, reciprocal,
               broadcast via K=1 matmul, normalize on psum evict
  - AV:        U[d,q] = Vtok[k,d].T @ E[k,q]     (bf16)
  - out:       out[t,o] = attn'[f,t].T @ WoT[f,o]  (partial; host sums)
"""
import math
import numpy as np
import ml_dtypes

import concourse.bacc as bacc
import concourse.tile as tile
from concourse import mybir
from concourse.bass_utils import run_bass_kernel_spmd

F32 = mybir.dt.float32
F32R = mybir.dt.float32r
BF16 = mybir.dt.bfloat16

# Compute dtypes (bf16 matmuls stream at 1 cyc/row; f32r takes 2 passes)
DT_PROJ = BF16     # x / Wq / Wk / Wv and the QKV projection matmuls
DT_QK = BF16       # Q'/K' after rope -> scores matmul
DT_ATT = BF16      # attn' and WoT -> output projection matmul
NP_PROJ = ml_dtypes.bfloat16 if DT_PROJ == BF16 else np.float32
NP_ATT = ml_dtypes.bfloat16 if DT_ATT == BF16 else np.float32

P = 128
B, S, H = 2, 2048, 4096
T = B * S
DK = 128                     # head dim
NHL = 4                      # q heads per core
FL = NHL * DK                # 512 local q features
TB = 512                     # phase-1 token block
NTB = T // TB                # 8
NA = H // P                  # 32 contraction tiles
QBS = 512                    # attention q-block
NQB = S // QBS               # 4 q-blocks per (batch, head)
NKT = S // P                 # 16 k-tiles per batch
SCALE = 1.0 / math.sqrt(DK)
NOB = H // 512               # 8 output column blocks
NTT = T // P                 # 32 output row tiles

_NC_CACHE = {}


def build():
    nc = bacc.Bacc(None, target_bir_lowering=False)

    xt = nc.dram_tensor("xt", [H, T], DT_PROJ, kind="ExternalInput")
    wqt = nc.dram_tensor("wqt", [H, FL], DT_PROJ, kind="ExternalInput")
    wkt = nc.dram_tensor("wkt", [H, DK], DT_PROJ, kind="ExternalInput")
    wvt = nc.dram_tensor("wvt", [H, DK], DT_PROJ, kind="ExternalInput")
    wot = nc.dram_tensor("wot", [FL, H], DT_ATT, kind="ExternalInput")
    cost = nc.dram_tensor("cost", [P, S], F32, kind="ExternalInput")
    sints = nc.dram_tensor("sints", [P, S], F32, kind="ExternalInput")
    trimask = nc.dram_tensor("trimask", [P, P], BF16, kind="ExternalInput")
    onesc = nc.dram_tensor("onesc", [P, 1], BF16, kind="ExternalInput")
    onesr = nc.dram_tensor("onesr", [1, P], F32R, kind="ExternalInput")
    out = nc.dram_tensor("out", [T, H], F32, kind="ExternalOutput")

    EXP = mybir.ActivationFunctionType.Exp

    with nc.allow_low_precision(reason="attention compute dtypes are "
                                       "deliberately reduced"), \
         tile.TileContext(nc) as tc:
        with tc.tile_pool(name="const", bufs=1) as cp, \
             tc.tile_pool(name="dram", bufs=1, space="DRAM") as dp:
            cos_sb = cp.tile([P, S], F32)
            sin_sb = cp.tile([P, S], F32)
            tri_sb = cp.tile([P, P], BF16)
            oc_sb = cp.tile([P, 1], BF16)
            or_sb = cp.tile([1, P], F32R)
            nc.sync.dma_start(out=cos_sb, in_=cost[:, :])
            nc.sync.dma_start(out=sin_sb, in_=sints[:, :])
            nc.sync.dma_start(out=tri_sb, in_=trimask[:, :])
            nc.sync.dma_start(out=oc_sb, in_=onesc[:, :])
            nc.sync.dma_start(out=or_sb, in_=onesr[:, :])

            # per-batch scratch so phase 2 (batch 0) overlaps phase 1 (batch 1)
            q_scr = [dp.tile([FL, S], DT_QK, name=f"qscr{b}") for b in range(B)]
            k_scr = [dp.tile([DK, S], DT_QK, name=f"kscr{b}") for b in range(B)]
            v_scr = [dp.tile([DK, S], BF16, name=f"vscr{b}") for b in range(B)]

            # Long-lived phase-2 SBUF pools allocated BEFORE phase-1 pools
            # so their loads don't carry WAR deps on phase-1 weight reads.
            apctx = tc.tile_pool(name="attn", bufs=1)
            ap = apctx.__enter__()
            attn_sb = [[ap.tile([P, S], DT_ATT, name=f"attn{b}_{h}")
                        for h in range(NHL)] for b in range(B)]
            p2ctx = tc.tile_pool(name="p2", bufs=1)
            p2 = p2ctx.__enter__()
            p2qctx = tc.tile_pool(name="p2q", bufs=2)
            p2q = p2qctx.__enter__()
            p2ectx = tc.tile_pool(name="p2e", bufs=4)
            p2e = p2ectx.__enter__()

            # ---------------- Phase 1: QKV projection + RoPE ----------------
            with tc.tile_pool(name="wq", bufs=1) as wqp, \
                 tc.tile_pool(name="xp", bufs=6) as xp, \
                 tc.tile_pool(name="rp", bufs=1) as rp, \
                 tc.tile_pool(name="ps1", bufs=1, space="PSUM") as ps1:
                wq_sb = wqp.tile([P, NA * FL], DT_PROJ)
                wk_sb = wqp.tile([P, NA * DK], DT_PROJ)
                wv_sb = wqp.tile([P, NA * DK], DT_PROJ)
                for a in range(NA):
                    nc.sync.dma_start(out=wq_sb[:, a * FL:(a + 1) * FL],
                                      in_=wqt[a * P:(a + 1) * P, :])
                    nc.sync.dma_start(out=wk_sb[:, a * DK:(a + 1) * DK],
                                      in_=wkt[a * P:(a + 1) * P, :])
                    nc.sync.dma_start(out=wv_sb[:, a * DK:(a + 1) * DK],
                                      in_=wvt[a * P:(a + 1) * P, :])

                for tb in range(NTB):
                    bi = (tb * TB) // S
                    s0 = (tb * TB) % S
                    psq = [ps1.tile([P, TB], F32, name=f"psq{j}_{tb}",
                                    tag=f"psq{j}") for j in range(NHL)]
                    psk = ps1.tile([P, TB], F32, name=f"psk_{tb}", tag="psk")
                    psv = ps1.tile([P, TB], F32, name=f"psv_{tb}", tag="psv")
                    for a in range(NA):
                        xt_t = xp.tile([P, TB], DT_PROJ, name=f"x_{tb}_{a}",
                                       tag="xt")
                        nc.sync.dma_start(
                            out=xt_t,
                            in_=xt[a * P:(a + 1) * P, tb * TB:(tb + 1) * TB])
                        st, sp = (a == 0), (a == NA - 1)
                        nc.tensor.matmul(psk, wk_sb[:, a * DK:(a + 1) * DK],
                                         xt_t, start=st, stop=sp)
                        nc.tensor.matmul(psv, wv_sb[:, a * DK:(a + 1) * DK],
                                         xt_t, start=st, stop=sp)
                        for j in range(NHL):
                            nc.tensor.matmul(
                                psq[j],
                                wq_sb[:, a * FL + j * DK:a * FL + (j + 1) * DK],
                                xt_t, start=st, stop=sp)

                    # Evict all 6 psum banks with a single ACT reader each so
                    # the next t-block's matmuls can reclaim banks at once.
                    evs = []
                    plan = [(psk, k_scr, 0, nc.scalar),
                            (psq[0], q_scr, 0, nc.vector),
                            (psq[1], q_scr, P, nc.scalar),
                            (psq[2], q_scr, 2 * P, nc.vector),
                            (psq[3], q_scr, 3 * P, nc.scalar)]
                    for idx, (src, scr, r0, eng) in enumerate(plan):
                        qc = rp.tile([P, TB], F32, name=f"qc_{tb}_{idx}",
                                     tag="qc", bufs=7)
                        if eng is nc.scalar:
                            nc.scalar.copy(qc, src)
                        else:
                            nc.vector.tensor_copy(qc, src)
                        if idx == 0:
                            vb = rp.tile([P, TB], BF16, name=f"vb_{tb}",
                                         tag="vb", bufs=2)
                            nc.vector.tensor_copy(vb, psv)
                            nc.scalar.dma_start(
                                out=v_scr[bi][:, s0:s0 + TB], in_=vb)
                        evs.append((qc, scr, r0))

                    # RoPE chains (read only SBUF now; muls in place)
                    for qc, scr, r0 in evs:
                        sw = rp.tile([P, TB], F32, name=f"sw_{tb}_{r0}",
                                     tag="sw", bufs=7)
                        nc.scalar.dma_start(out=sw[0:64, :], in_=qc[64:128, :])
                        nc.scalar.dma_start(out=sw[64:128, :], in_=qc[0:64, :])
                        nc.vector.tensor_mul(qc, qc, cos_sb[:, s0:s0 + TB])
                        nc.vector.tensor_mul(sw, sw, sin_sb[:, s0:s0 + TB])
                        qf = rp.tile([P, TB], DT_QK, name=f"qf_{tb}_{r0}",
                                     tag="qf", bufs=7)
                        nc.vector.tensor_add(qf, qc, sw)
                        nc.scalar.dma_start(
                            out=scr[bi][r0:r0 + P, s0:s0 + TB], in_=qf)



            # ---------------- Phase 2+3 ----------------
            if True:
                # ---------------- Phase 2: causal attention ----------------
                with tc.tile_pool(name="ps2s", bufs=3, space="PSUM") as ps2s, \
                     tc.tile_pool(name="ps2u", bufs=2, space="PSUM") as ps2u, \
                     tc.tile_pool(name="ps2r", bufs=1, space="PSUM") as ps2r:
                    for b in range(B):
                        kb_sb = p2q.tile([P, S], DT_QK, name=f"kb_{b}",
                                         tag="kb")
                        nc.sync.dma_start(out=kb_sb, in_=k_scr[b][:, :])
                        vtk = p2q.tile([P, NKT, P], BF16, name=f"vt_{b}",
                                       tag="vtk")
                        nc.sync.dma_start_transpose(vtk, v_scr[b][:, :])
                        for h in range(NHL):
                            qh_sb = p2q.tile([P, S], DT_QK, name=f"q_{b}_{h}",
                                             tag="qh")
                            nc.sync.dma_start(out=qh_sb,
                                              in_=q_scr[b][h * P:(h + 1) * P, :])
                            if b == 0 and h == 0:
                                pending, mm_since = None, 0

                            def make_norm(b, h, qb, u_ps, d_ps):
                                def norm():
                                    rf_sb = p2.tile([1, QBS], F32,
                                                    name=f"rf_{b}_{h}_{qb}",
                                                    tag="rf", bufs=2)
                                    nc.vector.reciprocal_approx_fast(rf_sb,
                                                                     d_ps)
                                    r_sb = p2.tile([1, QBS], F32R,
                                                   name=f"r_{b}_{h}_{qb}",
                                                   tag="r", bufs=2)
                                    nc.vector.tensor_copy(r_sb, rf_sb)
                                    rb_ps = ps2r.tile([P, QBS], F32,
                                                      name=f"rb_{b}_{h}_{qb}",
                                                      tag="rb")
                                    nc.tensor.matmul(rb_ps, or_sb, r_sb,
                                                     start=True, stop=True)
                                    rb_sb = p2.tile([P, QBS], F32,
                                                    name=f"rs_{b}_{h}_{qb}",
                                                    tag="rs", bufs=2)
                                    nc.scalar.copy(rb_sb, rb_ps)
                                    nc.vector.tensor_mul(
                                        attn_sb[b][h][:, qb * QBS:
                                                      (qb + 1) * QBS],
                                        u_ps, rb_sb)
                                return norm

                            for qb in range(NQB):
                                nkt = 4 * qb + 4
                                u_ps = ps2u.tile([P, QBS], F32,
                                                 name=f"u_{b}_{h}_{qb}",
                                                 tag="u")
                                d_ps = ps2u.tile([1, QBS], F32,
                                                 name=f"d_{b}_{h}_{qb}",
                                                 tag="d")
                                def emit_av(kt, e_sb, lo):
                                    st, sp = (kt == 0), (kt == nkt - 1)
                                    nc.tensor.matmul(u_ps[:, lo:], vtk[:, kt, :],
                                                     e_sb[:, lo:],
                                                     start=st, stop=sp,
                                                     skip_group_check=True)
                                    nc.tensor.matmul(d_ps[:, lo:], oc_sb,
                                                     e_sb[:, lo:],
                                                     start=st, stop=sp,
                                                     skip_group_check=True)
                                av_fifo = []
                                for kt in range(nkt):
                                    s_ps = ps2s.tile(
                                        [P, QBS], F32,
                                        name=f"s_{b}_{h}_{qb}_{kt}", tag="s")
                                    m = kt - 4 * qb
                                    lo = m * P if m > 0 else 0
                                    nc.tensor.matmul(
                                        s_ps[:, lo:],
                                        kb_sb[:, kt * P:(kt + 1) * P],
                                        qh_sb[:, qb * QBS + lo:
                                              (qb + 1) * QBS],
                                        start=True, stop=True)
                                    e_sb = p2e.tile(
                                        [P, QBS], BF16,
                                        name=f"e_{b}_{h}_{qb}_{kt}", tag="e")
                                    nc.scalar.activation(e_sb[:, lo:],
                                                         s_ps[:, lo:], EXP,
                                                         scale=SCALE)
                                    if m >= 0:
                                        nc.vector.tensor_mul(
                                            e_sb[:, m * P:(m + 1) * P],
                                            e_sb[:, m * P:(m + 1) * P],
                                            tri_sb)
                                    if len(av_fifo) >= 2:
                                        emit_av(*av_fifo.pop(0))
                                    av_fifo.append((kt, e_sb, lo))
                                    mm_since += 3
                                    if pending is not None and mm_since >= 12:
                                        pending()
                                        pending = None
                                for args in av_fifo:
                                    emit_av(*args)
                                pending = make_norm(b, h, qb, u_ps, d_ps)
                                mm_since = 0
                    pending()

                p2ectx.__exit__(None, None, None)
                p2qctx.__exit__(None, None, None)
                p2ctx.__exit__(None, None, None)

                # ---------------- Phase 3: output projection ----------------
                with tc.tile_pool(name="p3w", bufs=2) as p3w, \
                     tc.tile_pool(name="p3o", bufs=4) as p3o, \
                     tc.tile_pool(name="ps3", bufs=4, space="PSUM") as ps3:
                    wo_tiles = {}

                    def load_wo(ob):
                        wo_sb = p3w.tile([P, NHL, 512], DT_ATT,
                                         name=f"wo_{ob}", tag="wo")
                        for j in range(NHL):
                            nc.sync.dma_start(
                                out=wo_sb[:, j, :],
                                in_=wot[j * P:(j + 1) * P,
                                        ob * 512:(ob + 1) * 512])
                        wo_tiles[ob] = wo_sb

                    load_wo(0)
                    for ob in range(NOB):
                        wo_sb = wo_tiles.pop(ob)
                        for tt in range(NTT):
                            if tt == 4 and ob + 1 < NOB:
                                load_wo(ob + 1)
                            bt, st_ = tt // (S // P), (tt % (S // P)) * P
                            o_ps = ps3.tile([P, 512], F32,
                                            name=f"o_{ob}_{tt}", tag="o")
                            for j in range(NHL):
                                nc.tensor.matmul(
                                    o_ps,
                                    attn_sb[bt][j][:, st_:st_ + P],
                                    wo_sb[:, j, :],
                                    start=(j == 0), stop=(j == NHL - 1))
                            o_sb = p3o.tile([P, 512], F32,
                                            name=f"os_{ob}_{tt}", tag="os")
                            nc.vector.tensor_copy(o_sb, o_ps)
                            nc.sync.dma_start(
                                out=out[tt * P:(tt + 1) * P,
                                        ob * 512:(ob + 1) * 512],
                                in_=o_sb)
                apctx.__exit__(None, None, None)

    nc.compile()
    return nc


def _prep_inputs(hidden_states, Wq, Wk, Wv, Wo, cos, sin):
    hs = np.asarray(hidden_states, dtype=np.float32)
    Wq = np.asarray(Wq, dtype=np.float32)
    Wk = np.asarray(Wk, dtype=np.float32)
    Wv = np.asarray(Wv, dtype=np.float32)
    Wo = np.asarray(Wo, dtype=np.float32)
    cos = np.asarray(cos, dtype=np.float32)
    sin = np.asarray(sin, dtype=np.float32)

    xt = np.ascontiguousarray(hs.reshape(T, H).T).astype(NP_PROJ)
    cosT = np.ascontiguousarray(cos.T)
    sinT = np.ascontiguousarray(sin.T)
    sints = np.ascontiguousarray(
        np.concatenate([-sinT[:64], sinT[64:]], axis=0))
    kq = np.arange(P)
    trim = (kq[None, :] >= kq[:, None]).astype(ml_dtypes.bfloat16)
    onesc = np.ones((P, 1), dtype=ml_dtypes.bfloat16)
    onesr = np.ones((1, P), dtype=np.float32)

    in_maps = []
    for c in range(8):
        in_maps.append({
            "xt": xt,
            "wqt": np.ascontiguousarray(
                Wq[c * FL:(c + 1) * FL, :].T).astype(NP_PROJ),
            "wkt": np.ascontiguousarray(
                Wk[c * DK:(c + 1) * DK, :].T).astype(NP_PROJ),
            "wvt": np.ascontiguousarray(
                Wv[c * DK:(c + 1) * DK, :].T).astype(NP_PROJ),
            "wot": np.ascontiguousarray(
                Wo[:, c * FL:(c + 1) * FL].T).astype(NP_ATT),
            "cost": cosT,
            "sints": sints,
            "trimask": trim,
            "onesc": onesc,
            "onesr": onesr,
        })
    return in_maps


def kernel(hidden_states, Wq, Wk, Wv, Wo, cos, sin, _run_kwargs=None):
    in_maps = _prep_inputs(hidden_states, Wq, Wk, Wv, Wo, cos, sin)
    if "nc" not in _NC_CACHE:
        _NC_CACHE["nc"] = build()
    nc = _NC_CACHE["nc"]
    kw = _run_kwargs or {}
    res = run_bass_kernel_spmd(nc, in_maps, core_ids=list(range(8)), **kw)
    acc = np.zeros((T, H), dtype=np.float64)
    for c in range(8):
        acc += np.asarray(res.results[c]["out"], dtype=np.float64)
    out = acc.astype(np.float32).reshape(B, S, H)
    if kw:
        _NC_CACHE["last_results"] = res
    return out


# revision 13
# speedup vs baseline: 1.4615x; 1.0369x over previous
"""Trainium2 Bass kernel for Llama GQA attention (B=2, S=2048, H=4096,
32 Q heads / 8 KV heads, head_dim 128, RoPE, causal).

Sharding: tensor-parallel by head across 8 cores. Core c owns Q heads
[4c..4c+3] and KV head c. Each core computes its Q/K/V projections,
RoPE, causal attention, and a partial output projection over its 512
attention features; the host sums the 8 partial outputs.

Device layout is feature-major ([feature, token]) throughout:
  - QKV proj:  Q'[f,t] (psum) = sum_h WqT[h,f].T @ xT[h,t]     (bf16)
  - RoPE:      q*cos + swap_halves(q)*sign*sin  (DVE + DMA swap)
  - scores:    S.T[k,q] = K'[d,k].T @ Q'[d,q]   (softmax over partition)
  - softmax:   exp on ACT (no max subtraction; scores are O(10)),
               denominator via ones-column matmul, fast reciprocal,
               K=1 broadcast matmul, normalize fused into psum evict
  - AV:        U[d,q] = Vtok[k,d].T @ E[k,q]    (bf16, causal-sliced)
  - out:       out[t,o] = attn'[f,t].T @ WoT[f,o]  (partial; host sums)

Batch-0's output projection is interleaved into batch-1's attention so
the PE chews o-proj matmuls while ACT runs the exp stream.
"""
import math
import numpy as np
import ml_dtypes

import concourse.bacc as bacc
import concourse.tile as tile
from concourse import mybir
from concourse.bass_utils import run_bass_kernel_spmd

F32 = mybir.dt.float32
F32R = mybir.dt.float32r
BF16 = mybir.dt.bfloat16

DT_PROJ = BF16
DT_QK = BF16
DT_ATT = BF16
NP_PROJ = ml_dtypes.bfloat16 if DT_PROJ == BF16 else np.float32
NP_ATT = ml_dtypes.bfloat16 if DT_ATT == BF16 else np.float32

P = 128
B, S, H = 2, 2048, 4096
T = B * S
DK = 128
NHL = 4
FL = NHL * DK
TB = 512
NTB = T // TB
NA = H // P
QBS = 512
NQB = S // QBS
NKT = S // P
SCALE = 1.0 / math.sqrt(DK)
NOB = H // 512
NTPB = S // P                # 16 output row tiles per batch

_NC_CACHE = {}


def build():
    nc = bacc.Bacc(None, target_bir_lowering=False)

    xt = nc.dram_tensor("xt", [H, T], DT_PROJ, kind="ExternalInput")
    wqt = nc.dram_tensor("wqt", [H, FL], DT_PROJ, kind="ExternalInput")
    wkt = nc.dram_tensor("wkt", [H, DK], DT_PROJ, kind="ExternalInput")
    wvt = nc.dram_tensor("wvt", [H, DK], DT_PROJ, kind="ExternalInput")
    wot = nc.dram_tensor("wot", [FL, H], DT_ATT, kind="ExternalInput")
    cost = nc.dram_tensor("cost", [P, S], F32, kind="ExternalInput")
    sints = nc.dram_tensor("sints", [P, S], F32, kind="ExternalInput")
    trimask = nc.dram_tensor("trimask", [P, P], BF16, kind="ExternalInput")
    onesc = nc.dram_tensor("onesc", [P, 1], BF16, kind="ExternalInput")
    out = nc.dram_tensor("out", [T, H], F32, kind="ExternalOutput")

    EXP = mybir.ActivationFunctionType.Exp

    with nc.allow_low_precision(reason="attention compute dtypes are "
                                       "deliberately reduced"), \
         tile.TileContext(nc) as tc:
        with tc.tile_pool(name="const", bufs=1) as cp, \
             tc.tile_pool(name="dram", bufs=1, space="DRAM") as dp, \
             tc.tile_pool(name="attn", bufs=1) as ap, \
             tc.tile_pool(name="p2", bufs=1) as p2, \
             tc.tile_pool(name="p2q", bufs=2) as p2q, \
             tc.tile_pool(name="p2e", bufs=5) as p2e, \
             tc.tile_pool(name="p3w", bufs=2) as p3w, \
             tc.tile_pool(name="p3o", bufs=4) as p3o:
            cos_sb = cp.tile([P, S], F32)
            sin_sb = cp.tile([P, S], F32)
            tri_sb = cp.tile([P, P], BF16)
            oc_sb = cp.tile([P, 1], BF16)
            nc.sync.dma_start(out=cos_sb, in_=cost[:, :])
            nc.sync.dma_start(out=sin_sb, in_=sints[:, :])
            nc.sync.dma_start(out=tri_sb, in_=trimask[:, :])
            nc.sync.dma_start(out=oc_sb, in_=onesc[:, :])

            attn_sb = [[ap.tile([P, S], DT_ATT, name=f"attn{b}_{h}")
                        for h in range(NHL)] for b in range(B)]
            q_scr = [dp.tile([FL, S], DT_QK, name=f"qscr{b}") for b in range(B)]
            k_scr = [dp.tile([DK, S], DT_QK, name=f"kscr{b}") for b in range(B)]
            v_scr = [dp.tile([DK, S], BF16, name=f"vscr{b}") for b in range(B)]

            # ---------------- Phase 1: QKV projection + RoPE ----------------
            with tc.tile_pool(name="wq", bufs=1) as wqp, \
                 tc.tile_pool(name="xp", bufs=6) as xp, \
                 tc.tile_pool(name="rp", bufs=1) as rp, \
                 tc.tile_pool(name="ps1", bufs=1, space="PSUM") as ps1:
                wq_sb = wqp.tile([P, NA * FL], DT_PROJ)
                wk_sb = wqp.tile([P, NA * DK], DT_PROJ)
                wv_sb = wqp.tile([P, NA * DK], DT_PROJ)
                for a in range(NA):
                    nc.sync.dma_start(out=wq_sb[:, a * FL:(a + 1) * FL],
                                      in_=wqt[a * P:(a + 1) * P, :])
                    nc.sync.dma_start(out=wk_sb[:, a * DK:(a + 1) * DK],
                                      in_=wkt[a * P:(a + 1) * P, :])
                    nc.sync.dma_start(out=wv_sb[:, a * DK:(a + 1) * DK],
                                      in_=wvt[a * P:(a + 1) * P, :])

                for tb in range(NTB):
                    bi = (tb * TB) // S
                    s0 = (tb * TB) % S
                    psq = [ps1.tile([P, TB], F32, name=f"psq{j}_{tb}",
                                    tag=f"psq{j}") for j in range(NHL)]
                    psk = ps1.tile([P, TB], F32, name=f"psk_{tb}", tag="psk")
                    psv = ps1.tile([P, TB], F32, name=f"psv_{tb}", tag="psv")
                    for a in range(NA):
                        xt_t = xp.tile([P, TB], DT_PROJ, name=f"x_{tb}_{a}",
                                       tag="xt")
                        nc.sync.dma_start(
                            out=xt_t,
                            in_=xt[a * P:(a + 1) * P, tb * TB:(tb + 1) * TB])
                        st, sp = (a == 0), (a == NA - 1)
                        nc.tensor.matmul(psk, wk_sb[:, a * DK:(a + 1) * DK],
                                         xt_t, start=st, stop=sp)
                        nc.tensor.matmul(psv, wv_sb[:, a * DK:(a + 1) * DK],
                                         xt_t, start=st, stop=sp)
                        for j in range(NHL):
                            nc.tensor.matmul(
                                psq[j],
                                wq_sb[:, a * FL + j * DK:a * FL + (j + 1) * DK],
                                xt_t, start=st, stop=sp)

                    # evict psum banks (one reader each, split ACT/DVE; K
                    # first since the next t-block's matmuls demand it first)
                    evs = []
                    plan = [(psk, k_scr, 0, nc.scalar),
                            (psq[0], q_scr, 0, nc.vector),
                            (psq[1], q_scr, P, nc.scalar),
                            (psq[2], q_scr, 2 * P, nc.vector),
                            (psq[3], q_scr, 3 * P, nc.scalar)]
                    for idx, (src, scr, r0, eng) in enumerate(plan):
                        qc = rp.tile([P, TB], F32, name=f"qc_{tb}_{idx}",
                                     tag="qc", bufs=7)
                        if eng is nc.scalar:
                            nc.scalar.copy(qc, src)
                        else:
                            nc.vector.tensor_copy(qc, src)
                        if idx == 0:
                            vb = rp.tile([P, TB], BF16, name=f"vb_{tb}",
                                         tag="vb", bufs=2)
                            nc.vector.tensor_copy(vb, psv)
                            nc.scalar.dma_start(
                                out=v_scr[bi][:, s0:s0 + TB], in_=vb)
                        evs.append((qc, scr, r0))

                    # RoPE chains (SBUF only; eviction DMAs ride the ACT
                    # HWDGE ring so they never block the x-load stream)
                    for qc, scr, r0 in evs:
                        sw = rp.tile([P, TB], F32, name=f"sw_{tb}_{r0}",
                                     tag="sw", bufs=7)
                        nc.scalar.dma_start(out=sw[0:64, :], in_=qc[64:128, :])
                        nc.scalar.dma_start(out=sw[64:128, :], in_=qc[0:64, :])
                        nc.vector.tensor_mul(qc, qc, cos_sb[:, s0:s0 + TB])
                        nc.vector.tensor_mul(sw, sw, sin_sb[:, s0:s0 + TB])
                        qf = rp.tile([P, TB], DT_QK, name=f"qf_{tb}_{r0}",
                                     tag="qf", bufs=7)
                        nc.vector.tensor_add(qf, qc, sw)
                        nc.scalar.dma_start(
                            out=scr[bi][r0:r0 + P, s0:s0 + TB], in_=qf)

            # ------------- Phase 2 + interleaved output projection ----------
            with tc.tile_pool(name="ps2s", bufs=3, space="PSUM") as ps2s, \
                 tc.tile_pool(name="ps2u", bufs=2, space="PSUM") as ps2u:
                wo_tiles = {}

                def load_wo(ob):
                    wo_sb = p3w.tile([P, NHL, 512], DT_ATT,
                                     name=f"wo_{ob}", tag="wo")
                    for j in range(NHL):
                        nc.sync.dma_start(
                            out=wo_sb[:, j, :],
                            in_=wot[j * P:(j + 1) * P,
                                    ob * 512:(ob + 1) * 512])
                    wo_tiles[ob] = wo_sb

                def emit_otile(bt, ob, ti):
                    if ob not in wo_tiles:
                        load_wo(ob)
                    if ti == 4 and ob + 1 < NOB and (ob + 1) not in wo_tiles:
                        load_wo(ob + 1)
                    tt = bt * NTPB + ti
                    o_ps = ps2u.tile([P, 512], F32,
                                     name=f"o_{bt}_{ob}_{ti}", tag="u",
                                     bufs=3)
                    for j in range(NHL):
                        nc.tensor.matmul(
                            o_ps, attn_sb[bt][j][:, ti * P:(ti + 1) * P],
                            wo_tiles[ob][:, j, :],
                            start=(j == 0), stop=(j == NHL - 1))
                    o_sb = p3o.tile([P, 512], F32,
                                    name=f"os_{bt}_{ob}_{ti}", tag="os")
                    nc.vector.tensor_copy(o_sb, o_ps)
                    nc.sync.dma_start(
                        out=out[tt * P:(tt + 1) * P, ob * 512:(ob + 1) * 512],
                        in_=o_sb)
                    if ti == NTPB - 1:
                        wo_tiles.pop(ob, None)

                def make_norm(b, h, qb, u_ps, d_ps):
                    def norm():
                        rf_sb = p2.tile([1, QBS], F32,
                                        name=f"rf_{b}_{h}_{qb}",
                                        tag="rf", bufs=2)
                        nc.vector.reciprocal_approx_fast(rf_sb, d_ps)
                        rb_sb = p2.tile([P, QBS], F32,
                                        name=f"rs_{b}_{h}_{qb}",
                                        tag="rs", bufs=2)
                        nc.gpsimd.partition_broadcast(rb_sb, rf_sb)
                        nc.vector.tensor_mul(
                            attn_sb[b][h][:, qb * QBS:(qb + 1) * QBS],
                            u_ps, rb_sb)
                    return norm

                # batch-0 o-proj tiles drip-fed into batch-1's attention
                inter = [(0, ob, ti) for ob in range(NOB)
                         for ti in range(NTPB)]
                inter_pos = 0

                pending = None
                mm_since = 0
                for b in range(B):
                    kb_sb = p2q.tile([P, S], DT_QK, name=f"kb_{b}", tag="kb")
                    nc.sync.dma_start(out=kb_sb, in_=k_scr[b][:, :])
                    vtk = p2q.tile([P, NKT, P], BF16, name=f"vt_{b}",
                                   tag="vtk")
                    nc.sync.dma_start_transpose(vtk, v_scr[b][:, :])
                    for h in range(NHL):
                        qh_sb = p2q.tile([P, S], DT_QK, name=f"q_{b}_{h}",
                                         tag="qh")
                        nc.sync.dma_start(out=qh_sb,
                                          in_=q_scr[b][h * P:(h + 1) * P, :])
                        for qb in range(NQB):
                            nkt = 4 * qb + 4
                            u_ps = ps2u.tile([P, QBS], F32,
                                             name=f"u_{b}_{h}_{qb}", tag="u",
                                             bufs=3)
                            d_ps = ps2u.tile([1, QBS], F32,
                                             name=f"d_{b}_{h}_{qb}", tag="d")

                            def emit_av(kt, e_sb, lo, u_ps=None, d_ps=None,
                                        nkt=None):
                                st, sp = (kt == 0), (kt == nkt - 1)
                                nc.tensor.matmul(u_ps[:, lo:], vtk[:, kt, :],
                                                 e_sb[:, lo:],
                                                 start=st, stop=sp,
                                                 skip_group_check=True)
                                nc.tensor.matmul(d_ps[:, lo:], oc_sb,
                                                 e_sb[:, lo:],
                                                 start=st, stop=sp,
                                                 skip_group_check=True)

                            av_fifo = []
                            for kt in range(nkt):
                                s_ps = ps2s.tile(
                                    [P, QBS], F32,
                                    name=f"s_{b}_{h}_{qb}_{kt}", tag="s")
                                m = kt - 4 * qb
                                lo = m * P if m > 0 else 0
                                nc.tensor.matmul(
                                    s_ps[:, lo:],
                                    kb_sb[:, kt * P:(kt + 1) * P],
                                    qh_sb[:, qb * QBS + lo:(qb + 1) * QBS],
                                    start=True, stop=True)
                                e_sb = p2e.tile(
                                    [P, QBS], BF16,
                                    name=f"e_{b}_{h}_{qb}_{kt}", tag="e")
                                nc.scalar.activation(e_sb[:, lo:],
                                                     s_ps[:, lo:], EXP,
                                                     scale=SCALE)
                                if m >= 0:
                                    nc.vector.tensor_mul(
                                        e_sb[:, m * P:(m + 1) * P],
                                        e_sb[:, m * P:(m + 1) * P],
                                        tri_sb)
                                if len(av_fifo) >= 3:
                                    a0 = av_fifo.pop(0)
                                    emit_av(*a0, u_ps=u_ps, d_ps=d_ps,
                                            nkt=nkt)
                                av_fifo.append((kt, e_sb, lo))
                                mm_since += 3
                                if pending is not None and mm_since >= 12:
                                    pending()
                                    pending = None
                            for a0 in av_fifo:
                                emit_av(*a0, u_ps=u_ps, d_ps=d_ps, nkt=nkt)
                            pending = make_norm(b, h, qb, u_ps, d_ps)
                            mm_since = 0

                            # drip batch-0 o-proj into batch-1's attention
                            if b == 1:
                                for _ in range(8):
                                    if inter_pos < len(inter):
                                        emit_otile(*inter[inter_pos])
                                        inter_pos += 1
                pending()

                # leftover batch-0 tiles, then all of batch 1
                while inter_pos < len(inter):
                    emit_otile(*inter[inter_pos])
                    inter_pos += 1
                wo_tiles.clear()
                for ob in range(NOB):
                    for ti in range(NTPB):
                        emit_otile(1, ob, ti)

    nc.compile()
    return nc


def _prep_inputs(hidden_states, Wq, Wk, Wv, Wo, cos, sin):
    hs = np.asarray(hidden_states, dtype=np.float32)
    Wq = np.asarray(Wq, dtype=np.float32)
    Wk = np.asarray(Wk, dtype=np.float32)
    Wv = np.asarray(Wv, dtype=np.float32)
    Wo = np.asarray(Wo, dtype=np.float32)
    cos = np.asarray(cos, dtype=np.float32)
    sin = np.asarray(sin, dtype=np.float32)

    xt = np.ascontiguousarray(hs.reshape(T, H).T).astype(NP_PROJ)
    cosT = np.ascontiguousarray(cos.T)
    sinT = np.ascontiguousarray(sin.T)
    sints = np.ascontiguousarray(
        np.concatenate([-sinT[:64], sinT[64:]], axis=0))
    kq = np.arange(P)
    trim = (kq[None, :] >= kq[:, None]).astype(ml_dtypes.bfloat16)
    onesc = np.ones((P, 1), dtype=ml_dtypes.bfloat16)

    in_maps = []
    for c in range(8):
        in_maps.append({
            "xt": xt,
            "wqt": np.ascontiguousarray(
                Wq[c * FL:(c + 1) * FL, :].T).astype(NP_PROJ),
            "wkt": np.ascontiguousarray(
                Wk[c * DK:(c + 1) * DK, :].T).astype(NP_PROJ),
            "wvt": np.ascontiguousarray(
                Wv[c * DK:(c + 1) * DK, :].T).astype(NP_PROJ),
            "wot": np.ascontiguousarray(
                Wo[:, c * FL:(c + 1) * FL].T).astype(NP_ATT),
            "cost": cosT,
            "sints": sints,
            "trimask": trim,
            "onesc": onesc,
        })
    return in_maps


def kernel(hidden_states, Wq, Wk, Wv, Wo, cos, sin, _run_kwargs=None):
    in_maps = _prep_inputs(hidden_states, Wq, Wk, Wv, Wo, cos, sin)
    if "nc" not in _NC_CACHE:
        _NC_CACHE["nc"] = build()
    nc = _NC_CACHE["nc"]
    kw = _run_kwargs or {}
    res = run_bass_kernel_spmd(nc, in_maps, core_ids=list(range(8)), **kw)
    acc = np.zeros((T, H), dtype=np.float64)
    for c in range(8):
        acc += np.asarray(res.results[c]["out"], dtype=np.float64)
    out = acc.astype(np.float32).reshape(B, S, H)
    if kw:
        _NC_CACHE["last_results"] = res
    return out
